# revision 4
# baseline (speedup 1.0000x reference)
"""ConvSP (SPH smoothing-kernel convolution) Trainium2 Bass kernel, v3.

Math (per batch b):
  out[o,i] = bias[o] + sum_k sum_j A_k[o,j] * relu(r2 - |x_i - x_j + off_k|^2)^3
  A_k[o,j] = knorm * sum_c weight[o,c,k] * dcoef[c,j],  dcoef = data / (invmass*density)

Wall-clock is dominated by the axon tunnel: ~40-80ms RTT (time-varying) plus
~75MB/s up / ~45MB/s down. v1 shipped 688KB/core (5.5MB total, payloads
replicated 4-8x). v3 ships each core ~83KB of UNIQUE bytes (666KB total) and
reconstructs everything else on device:

  - dcoef (bf16) + raw positions + tiny coeff tables form a per-batch blob,
    split 4 ways; an on-device HBM AllGather over replica groups
    [[0-3],[4-7]] rebuilds it on every core (NeuronLink, ~us).
  - wT*knorm (bf16, batch-independent) is split 8 ways and AllGathered
    across all cores.
  - t[j,i] = r2 - |x_i - x_j + off_k|^2 is a rank-4 bilinear form
      t = x_j*(2x_i) + y_j*(2y_i) + h_k[j]*1 + 1*row3_k[i]
      h_k[j]   = 2off_k.x_j - n2_j                     (j-side, per k)
      row3_k[i]= r2 - |off_k|^2 - n2_i - 2off_k.x_i    (i-side, per k)
    h/row3 for all 9 cells come from two tiny f32 matmuls against coeff
    tables [3,9]/[4,9]; n2/ones rows are computed on device (ACT square +
    DVE add / memset), so only x,y ship on the j-side. The i-side basis
    [2x_i,2y_i,n2_i] (6KB) ships per core (the i-block differs per core and
    uniform SPMD code cannot address it dynamically).
  - Output ships bf16 [64,512] per core (512KB total vs 1MB f32).
  - The jitted executable is AOT-compiled via fast_dispatch_compile (C++
    fast-path dispatch, saves ~2ms/call of Python dispatch overhead).

Device pipeline per core (one batch, one 512-wide i-block):
  A-phase: AT[j,o] per (k, j-chunk) = matmul(dcoef chunk, wT k-block) -> bf16.
  T-phase: one K=4 f32 matmul per [128j x 512i] tile of t; ACT computes t^2,
  DVE w = max(t,0)*t^2 (bf16), PE accumulates out += AT.T @ w over 144 tiles
  into PSUM (two 64-row halves, summed at the end).
"""

import os
import sys

import numpy as np

for _p in ("/opt/trn_rl_repo", "/root/.axon_site/_ro/trn_rl_repo"):
    if os.path.isdir(_p) and _p not in sys.path:
        sys.path.append(_p)

import ml_dtypes  # noqa: E402

import concourse.bass as bass  # noqa: E402
import concourse.mybir as mybir  # noqa: E402
import concourse.tile as tile  # noqa: E402

# ---------------------------------------------------------------- constants
NDIM = 2
KSIZE = (3, 3)
DILATION = (0.05, 0.05)
RADIUS = 0.1
C_IN = 64
C_OUT = 64
B = 2
N = 2048
NCELLS = 9
R2 = RADIUS * RADIUS
KNORM = 315.0 / (64.0 * np.pi * RADIUS**9)

NCORES = 8
IBLK = 512           # i-columns per core
NCHUNK = N // 128    # 16 j-chunks of 128
M_TOT = NCELLS * NCHUNK  # 144 (k, j-chunk) tiles

F32 = mybir.dt.float32
BF16 = mybir.dt.bfloat16

# ---- per-batch blob layout (f32 word offsets; bf16 bit-packed) ----------
SZ_DAT = C_IN * N // 2              # dcoef bf16 [64,2048] -> 65536 words
SZ_POS = 2 * N                      # x row | y row f32    -> 4096
SZ_CH = 3 * NCELLS                  # h coeffs  [3,9]      -> 27
SZ_C4 = 4 * NCELLS                  # row3 coeffs [4,9]    -> 36
OFF_DAT = 0
OFF_POS = OFF_DAT + SZ_DAT
OFF_CH = OFF_POS + SZ_POS
OFF_C4 = OFF_CH + SZ_CH
BLOB_RAW = OFF_C4 + SZ_C4           # 69695
GSLICE = -(-BLOB_RAW // (4 * 16)) * 16  # 17424 words per core (x16 aligned)
BLOB = 4 * GSLICE                   # 69696

SZ_WT = C_IN * NCELLS * C_OUT // 2  # wT bf16 [64,576] -> 18432 words
WSLICE = SZ_WT // NCORES            # 2304 words per core

SZ_BU = 3 * IBLK                    # per-core [2x,2y,n2]_i f32 -> 1536
PACK = GSLICE + WSLICE + SZ_BU      # 21264 f32 words = 83.1 KiB per core
OFF_W = GSLICE
OFF_BU = GSLICE + WSLICE

_cache: dict = {}


# ------------------------------------------------- TileContext drain patch
# The walrus in this container rejects the Tile tail-drain when it carries
# more than ~2 sem waits ("Too many sync wait commands"). Split the waits
# over extra sync-engine NOPs, one wait each.
def _patch_tile_drain():
    if getattr(tile.TileContext, "_drain_patched", False):
        return
    import bass_rust
    from concourse.vector_clock import ScopedClock

    def _drain_and_barrier(self, tick_clock, wait_clock):
        drain_inst = self.nc.sync.drain()
        wait_clock.add_sem_waits(
            drain_inst.ins, ScopedClock({None: tick_clock.global_clock})
        )
        si = drain_inst.ins.sync_info
        waits = list(si.on_wait) if si is not None else []
        if len(waits) > 1:
            si.on_wait = waits[:1]
            drain_inst.ins.sync_info = si
            for w in waits[1:]:
                n = self.nc.sync.nop(nofuse=True, hint="drain_wait_split")
                n.ins.sync_info = bass_rust.SyncInfo(on_wait=[w], on_update=[])
        self.nc.all_engine_barrier()
        popped = self.nc._tile_sem_poison_stack.pop()
        assert popped is self._sem_poison
        self.nc.clear_and_free_semaphores(list(self.sems.allocated().values()))
        self.nc.all_engine_barrier()

    tile.TileContext._drain_and_barrier = _drain_and_barrier
    tile.TileContext._drain_patched = True


# --------------------------------------------- sync-wait legalization pass
# This walrus rejects instructions carrying more than ~1-2 sem waits. After
# Tile scheduling, move excess waits onto same-engine NoOps inserted right
# before the over-subscribed instruction (engines execute their stream in
# order, so semantics are identical).
_WAIT_LIMIT = 1


def _split_sync_waits(nc, limit=_WAIT_LIMIT):
    cnt = 0
    for f in nc.m.functions:
        for bb in f.blocks:
            changed = False
            out = []
            for inst in bb.instructions:
                si = inst.sync_info
                waits = list(si.on_wait) if si is not None else []
                if len(waits) > limit:
                    keep = waits[-limit:]
                    excess = waits[:-limit]
                    for j in range(0, len(excess), limit):
                        n = mybir.InstNoOp(
                            name=f"waitsplit_{cnt}",
                            engine=inst.engine,
                            ins=[],
                            outs=[],
                            sync_info=mybir.SyncInfo(
                                on_wait=excess[j : j + limit], on_update=[]
                            ),
                        )
                        cnt += 1
                        nc.register_instruction(n, overwrite=True)
                        out.append(n)
                    si.on_wait = keep
                    inst.sync_info = si
                    changed = True
                out.append(inst)
            if changed:
                bb.instructions = out
    return cnt


# ------------------------------------------------------------- device build
def _build_nc():
    _patch_tile_drain()

    nc = bass.Bass(num_devices=NCORES)
    pack_d = nc.declare_dram_parameter("pack", [1, PACK], F32, isOutput=False)
    out_d = nc.declare_dram_parameter("out", [C_OUT, IBLK], BF16, isOutput=True)

    bu_v = pack_d[0:1, OFF_BU : OFF_BU + SZ_BU].rearrange(
        "a (p c) -> (a p) c", p=3
    )  # [3, 512] = [2x,2y,n2]_i

    from contextlib import ExitStack

    with tile.TileContext(nc) as tc, ExitStack() as ctx:
        dram = ctx.enter_context(tc.tile_pool(name="dram", bufs=1, space="DRAM"))
        const = ctx.enter_context(tc.tile_pool(name="const", bufs=1))
        qpool = ctx.enter_context(tc.tile_pool(name="q", bufs=4))
        wpool = ctx.enter_context(tc.tile_pool(name="w", bufs=4))
        apool = ctx.enter_context(tc.tile_pool(name="aps", bufs=2, space="PSUM"))
        tpool = ctx.enter_context(tc.tile_pool(name="t", bufs=2, space="PSUM"))
        opool = ctx.enter_context(tc.tile_pool(name="o", bufs=1, space="PSUM"))

        # ---- AllGather 1: per-batch blob (cores 0-3: b=0, cores 4-7: b=1)
        b_in = dram.tile([1, GSLICE], F32)
        flat = dram.tile([1, BLOB], F32)
        nc.gpsimd.dma_start(b_in[:], pack_d[0:1, 0:GSLICE])
        nc.gpsimd.collective_compute(
            "AllGather",
            mybir.AluOpType.bypass,
            replica_groups=[[0, 1, 2, 3], [4, 5, 6, 7]],
            ins=[b_in[:].opt()],
            outs=[flat[:].opt()],
        )
        # ---- AllGather 2: wT (batch-independent), 8-way
        w_in = dram.tile([1, WSLICE], F32)
        wgat = dram.tile([1, SZ_WT], F32)
        nc.gpsimd.dma_start(w_in[:], pack_d[0:1, OFF_W : OFF_W + WSLICE])
        nc.gpsimd.collective_compute(
            "AllGather",
            mybir.AluOpType.bypass,
            replica_groups=[[0, 1, 2, 3, 4, 5, 6, 7]],
            ins=[w_in[:].opt()],
            outs=[wgat[:].opt()],
        )

        dat_v = (
            flat[0:1, OFF_DAT : OFF_DAT + SZ_DAT]
            .bitcast(BF16)
            .rearrange("a (p c) -> (a p) c", p=C_IN)
        )  # [64, 2048] bf16
        xr_v = flat[0:1, OFF_POS : OFF_POS + N]            # [1, 2048]
        yr_v = flat[0:1, OFF_POS + N : OFF_POS + 2 * N]    # [1, 2048]
        ch_v = flat[0:1, OFF_CH : OFF_CH + SZ_CH].rearrange(
            "a (p c) -> (a p) c", p=3
        )  # [3, 9]
        c4_v = flat[0:1, OFF_C4 : OFF_C4 + SZ_C4].rearrange(
            "a (p c) -> (a p) c", p=4
        )  # [4, 9]
        wt_v = (
            wgat[0:1, :]
            .bitcast(BF16)
            .rearrange("a (p c) -> (a p) c", p=C_IN)
        )  # [64, 576] bf16

        datb = const.tile([C_IN, N], BF16)
        nc.sync.dma_start(datb[:], dat_v)
        wtb = const.tile([C_IN, NCELLS * C_OUT], BF16)
        nc.sync.dma_start(wtb[:], wt_v)

        # j-side basis rows (partition 0 each so ACT/DVE base-alignment holds)
        xr = const.tile([1, N], F32)
        nc.sync.dma_start(xr[:], xr_v)
        yr = const.tile([1, N], F32)
        nc.sync.dma_start(yr[:], yr_v)
        xsq = const.tile([1, N], F32)
        nc.scalar.square(xsq[:], xr[:])
        ysq = const.tile([1, N], F32)
        nc.scalar.square(ysq[:], yr[:])
        n2r = const.tile([1, N], F32)
        nc.vector.tensor_add(n2r[:], xsq[:], ysq[:])
        onesr = const.tile([1, N], F32)
        nc.vector.memset(onesr[:], 1.0)

        # matmul operands for the h / row3 coefficient matmuls
        bv_sb = const.tile([3, N], F32)     # [x, y, n2]_j
        nc.sync.dma_start(bv_sb[0:1, :], xr[:])
        nc.sync.dma_start(bv_sb[1:2, :], yr[:])
        nc.sync.dma_start(bv_sb[2:3, :], n2r[:])
        ch_sb = const.tile([3, NCELLS], F32)
        nc.sync.dma_start(ch_sb[:], ch_v)
        c4_sb = const.tile([4, NCELLS], F32)
        nc.sync.dma_start(c4_sb[:], c4_v)
        ub_sb = const.tile([4, IBLK], F32)  # [2x,2y,n2,1]_i basis
        nc.sync.dma_start(ub_sb[0:3, :], bu_v)
        nc.sync.dma_start(ub_sb[3:4, :], onesr[:, 0:IBLK])

        # V/U 4-row groups at 32-aligned partitions (PE tile-position rule).
        # Cell k lives at partition group g=k%4, column block cb=k//4.
        # V rows: [x_j, y_j, h_k[j], 1]   (x/y/1 k-independent, h per-k)
        # U rows: [2x_i, 2y_i, 1, row3_k[i]]
        vt = const.tile([128, 3 * N], F32)
        ut = const.tile([128, 3 * IBLK], F32)
        for k in range(NCELLS):
            g4, cb = 32 * (k % 4), k // 4
            nc.sync.dma_start(vt[g4 : g4 + 1, cb * N : (cb + 1) * N], xr[:])
            nc.sync.dma_start(vt[g4 + 1 : g4 + 2, cb * N : (cb + 1) * N], yr[:])
            nc.sync.dma_start(
                vt[g4 + 3 : g4 + 4, cb * N : (cb + 1) * N], onesr[:]
            )
            nc.sync.dma_start(
                ut[g4 : g4 + 2, cb * IBLK : (cb + 1) * IBLK], bu_v[0:2, :]
            )
            nc.sync.dma_start(
                ut[g4 + 2 : g4 + 3, cb * IBLK : (cb + 1) * IBLK],
                onesr[:, 0:IBLK],
            )

        # h[k, j] for all 9 cells: matmul against [x,y,n2] basis, 4 col chunks
        hstage = const.tile([NCELLS, N], F32)
        for cc in range(4):
            h_ps = opool.tile([NCELLS, 512], F32, tag="ops")
            nc.tensor.matmul(
                h_ps[:], ch_sb[:], bv_sb[:, cc * 512 : (cc + 1) * 512],
                start=True, stop=True,
            )
            nc.scalar.copy(hstage[:, cc * 512 : (cc + 1) * 512], h_ps[:])
        # row3[k, i] for the local i-block: one matmul
        r3stage = const.tile([NCELLS, IBLK], F32)
        r3_ps = opool.tile([NCELLS, IBLK], F32, tag="ops")
        nc.tensor.matmul(r3_ps[:], c4_sb[:], ub_sb[:], start=True, stop=True)
        nc.scalar.copy(r3stage[:], r3_ps[:])
        for k in range(NCELLS):
            g4, cb = 32 * (k % 4), k // 4
            nc.sync.dma_start(
                vt[g4 + 2 : g4 + 3, cb * N : (cb + 1) * N],
                hstage[k : k + 1, :],
            )
            nc.sync.dma_start(
                ut[g4 + 3 : g4 + 4, cb * IBLK : (cb + 1) * IBLK],
                r3stage[k : k + 1, :],
            )

        # A-phase: at[:, m*64:(m+1)*64] = AT chunk for m = k*16 + jc,
        # batched 8 matmuls per PSUM bank -> one ACT copy (f32->bf16) each.
        at = const.tile([128, M_TOT * C_OUT], BF16)
        for g in range(M_TOT // 8):
            a_ps = apool.tile([128, 512], F32)
            for r in range(8):
                m = g * 8 + r
                k, jc = divmod(m, NCHUNK)
                nc.tensor.matmul(
                    a_ps[:, r * 64 : (r + 1) * 64],
                    datb[:, jc * 128 : (jc + 1) * 128],
                    wtb[:, k * 64 : (k + 1) * 64],
                    start=True,
                    stop=True,
                )
            nc.scalar.copy(at[:, g * 512 : (g + 1) * 512], a_ps[:])

        # T-phase + main accumulation, software-pipelined: t-matmuls and
        # ACT/DVE cube for pair p run while PE accumulates mains of p-2.
        out_ps = opool.tile([128, IBLK], F32, tag="ops")
        pend = []  # (m, w_tile, half) awaiting the main matmul
        for gp in range(M_TOT // 2):
            t_ps = tpool.tile([128, 2 * IBLK], F32)
            for r in range(2):
                m = gp * 2 + r
                k, jc = divmod(m, NCHUNK)
                g4, cb = 32 * (k % 4), k // 4
                nc.tensor.matmul(
                    t_ps[:, r * IBLK : (r + 1) * IBLK],
                    vt[g4 : g4 + 4, cb * N + jc * 128 : cb * N + jc * 128 + 128],
                    ut[g4 : g4 + 4, cb * IBLK : (cb + 1) * IBLK],
                    start=True,
                    stop=True,
                    tile_position=(g4, 0),
                )
            q_t = qpool.tile([128, 2 * IBLK], BF16)
            nc.scalar.square(q_t[:], t_ps[:])
            w_t = wpool.tile([128, 2 * IBLK], BF16)
            nc.vector.scalar_tensor_tensor(
                w_t[:], t_ps[:], 0.0, q_t[:],
                op0=mybir.AluOpType.max, op1=mybir.AluOpType.mult,
            )
            pend.append((gp * 2, w_t, 0))
            pend.append((gp * 2 + 1, w_t, 1))
            while len(pend) > 4 or (gp == M_TOT // 2 - 1 and pend):
                m, w_tile, r = pend.pop(0)
                par = (m % 2) * C_OUT
                nc.tensor.matmul(
                    out_ps[par : par + C_OUT, :],
                    at[:, m * C_OUT : (m + 1) * C_OUT],
                    w_tile[:, r * IBLK : (r + 1) * IBLK],
                    start=(m < 2),
                    stop=(m >= M_TOT - 2),
                    skip_group_check=True,
                    tile_position=(0, par),
                )

        tmp_sb = const.tile([C_OUT, IBLK], F32)
        nc.scalar.copy(tmp_sb[:], out_ps[0:C_OUT, :])
        out_sb = const.tile([C_OUT, IBLK], BF16)
        nc.vector.tensor_add(out_sb[:], tmp_sb[:], out_ps[C_OUT:, :])
        nc.sync.dma_start(out_d[:], out_sb[:])
    _split_sync_waits(nc)
    return nc


# ------------------------------------------------------------- cached runner
def _get_runner():
    """Build (once) the jitted SPMD executable: pack [8, PACK] -> out bf16.

    Same bass_exec/PJRT machinery as bass_utils.run_bass_kernel_spmd under
    axon, but the executable is cached so repeat launches skip retracing,
    and AOT-compiled under fast_dispatch_compile (C++ fast-path dispatch).
    """
    if "runner" in _cache:
        return _cache["runner"]

    import jax
    from jax.sharding import Mesh, PartitionSpec

    from jax.experimental.shard_map import shard_map
    from concourse import bass2jax

    bass2jax.install_neuronx_cc_hook()

    nc = _build_nc()

    partition_name = (
        nc.partition_id_tensor.name if nc.partition_id_tensor else None
    )
    in_names, out_names, out_avals = [], [], []
    for alloc in nc.m.functions[0].allocations:
        if not isinstance(alloc, mybir.MemoryLocationSet):
            continue
        name = alloc.memorylocations[0].name
        if alloc.kind == "ExternalInput":
            if name != partition_name:
                in_names.append(name)
        elif alloc.kind == "ExternalOutput":
            out_avals.append(
                jax.core.ShapedArray(
                    tuple(alloc.tensor_shape), mybir.dt.np(alloc.dtype)
                )
            )
            out_names.append(name)
    assert in_names == ["pack"] and out_names == ["out"], (in_names, out_names)
    # "out" is deliberately NOT an operand: the kernel writes every element,
    # so no pre-zeroed donated buffer needs to ship through the tunnel.
    in_names_full = in_names + ([partition_name] if partition_name else [])

    def _body(pack):
        operands = [pack]
        if partition_name is not None:
            operands.append(bass2jax.partition_id_tensor())
        outs = bass2jax._bass_exec_p.bind(
            *operands,
            out_avals=tuple(out_avals),
            in_names=tuple(in_names_full),
            out_names=tuple(out_names),
            lowering_input_output_aliases=(),
            sim_require_finite=True,
            sim_require_nnan=True,
            nc=nc,
        )
        return outs[0]

    devices = jax.devices()[:NCORES]
    assert len(devices) == NCORES, f"need {NCORES} devices, got {len(devices)}"
    mesh = Mesh(np.asarray(devices), ("core",))

    def _make_jit():
        return jax.jit(
            shard_map(
                _body,
                mesh=mesh,
                in_specs=(PartitionSpec("core"),),
                out_specs=PartitionSpec("core"),
                check_rep=False,
            )
        )

    try:
        fn = bass2jax.fast_dispatch_compile(
            lambda: _make_jit()
            .lower(jax.ShapeDtypeStruct((NCORES, PACK), np.float32))
            .compile()
        )
    except Exception:
        fn = _make_jit()
    _cache["runner"] = fn
    return fn


# ------------------------------------------------------------ host wrapper
def _offsets():
    axes = [
        (np.arange(kk) - (kk - 1) / 2.0) * d for kk, d in zip(KSIZE, DILATION)
    ]
    grids = np.meshgrid(*axes, indexing="ij")
    return np.stack([g.reshape(-1) for g in grids], axis=-1).astype(np.float32)


def _prepare_pack(locs, data, density, weight):
    locs = np.asarray(locs, np.float32)
    data = np.asarray(data, np.float32)
    density = np.asarray(density, np.float32)
    weight = np.asarray(weight, np.float32)

    pos = locs[..., :NDIM]                        # [B,N,2]
    invmass = locs[..., NDIM]                     # [B,N]
    coef = 1.0 / (invmass * density)              # [B,N]
    dco = (data * coef[:, None, :]).astype(ml_dtypes.bfloat16)  # [B,64,N]
    # wT[c, k*64+o] = weight[o,c,k] * knorm
    wt = np.ascontiguousarray(weight.transpose(1, 2, 0) * np.float32(KNORM))
    wflat = wt.astype(ml_dtypes.bfloat16).reshape(C_IN, -1).view(np.float32)
    wflat = np.ascontiguousarray(wflat).ravel()   # [18432]
    offs = _offsets()                             # [9,2]

    ch = np.empty((3, NCELLS), np.float32)        # h = 2off.x_j - n2_j
    ch[0] = 2.0 * offs[:, 0]
    ch[1] = 2.0 * offs[:, 1]
    ch[2] = -1.0
    c4 = np.empty((4, NCELLS), np.float32)        # row3 on [2x,2y,n2,1]_i
    c4[0] = -offs[:, 0]
    c4[1] = -offs[:, 1]
    c4[2] = -1.0
    c4[3] = R2 - (offs**2).sum(1)

    pack = np.empty((NCORES, PACK), np.float32)
    blob = np.empty(BLOB, np.float32)
    blob[BLOB_RAW:] = 0.0
    for b in range(B):
        x, y = pos[b, :, 0], pos[b, :, 1]
        n2 = x * x + y * y
        blob[OFF_DAT : OFF_DAT + SZ_DAT] = (
            np.ascontiguousarray(dco[b]).view(np.float32).ravel()
        )
        blob[OFF_POS : OFF_POS + N] = x
        blob[OFF_POS + N : OFF_POS + 2 * N] = y
        blob[OFF_CH : OFF_CH + SZ_CH] = ch.ravel()
        blob[OFF_C4 : OFF_C4 + SZ_C4] = c4.ravel()
        bu = np.stack([2.0 * x, 2.0 * y, n2])     # [3, 2048]
        for q in range(4):
            c = b * 4 + q
            p = pack[c]
            p[0:GSLICE] = blob[q * GSLICE : (q + 1) * GSLICE]
            p[OFF_W : OFF_W + WSLICE] = wflat[c * WSLICE : (c + 1) * WSLICE]
            p[OFF_BU:] = bu[:, q * IBLK : (q + 1) * IBLK].ravel()
    return pack


def _launch(pack):
    fn = _get_runner()
    res = np.asarray(fn(pack))                    # [8*64, 512] bf16
    return res.reshape(NCORES, C_OUT, IBLK)


def _unpack_out(res, bias):
    out = np.empty((B, C_OUT, N), np.float32)
    for c in range(NCORES):
        b, q = divmod(c, 4)
        out[b][:, q * IBLK : (q + 1) * IBLK] = res[c]
    out += np.asarray(bias, np.float32)[None, :, None]
    return out


def kernel(locs, data, density, weight, bias):
    pack = _prepare_pack(locs, data, density, weight)
    return _unpack_out(_launch(pack), bias)


# -------------------------------------------------------------- benchmarking
def time_kernel(locs, data, density, weight, bias, iters=12):
    """Return (best_wall_s, per_call_s_list) for device launches.

    Host-side input prep runs once outside the loop; each timed iteration
    covers shipping the packed inputs to the 8 cores, executing, fetching,
    and unsharding the output.
    """
    import time

    kernel(locs, data, density, weight, bias)  # warm (compile)
    pack = _prepare_pack(locs, data, density, weight)
    times = []
    for _ in range(iters):
        t0 = time.perf_counter()
        _unpack_out(_launch(pack), bias)
        times.append(time.perf_counter() - t0)
    return min(times), times


# revision 5
# speedup vs baseline: 1.0203x; 1.0203x over previous
"""ConvSP (SPH smoothing-kernel convolution) Trainium2 Bass kernel, v3.

Math (per batch b):
  out[o,i] = bias[o] + sum_k sum_j A_k[o,j] * relu(r2 - |x_i - x_j + off_k|^2)^3
  A_k[o,j] = knorm * sum_c weight[o,c,k] * dcoef[c,j],  dcoef = data / (invmass*density)

Wall-clock is dominated by the axon tunnel: ~40-80ms RTT (time-varying) plus
~75MB/s up / ~45MB/s down. v1 shipped 688KB/core (5.5MB total, payloads
replicated 4-8x). v3 ships each core ~83KB of UNIQUE bytes (666KB total) and
reconstructs everything else on device:

  - dcoef (bf16) + raw positions + tiny coeff tables form a per-batch blob,
    split 4 ways; an on-device HBM AllGather over replica groups
    [[0-3],[4-7]] rebuilds it on every core (NeuronLink, ~us).
  - wT*knorm (bf16, batch-independent) is split 8 ways and AllGathered
    across all cores.
  - t[j,i] = r2 - |x_i - x_j + off_k|^2 is a rank-4 bilinear form
      t = x_j*(2x_i) + y_j*(2y_i) + h_k[j]*1 + 1*row3_k[i]
      h_k[j]   = 2off_k.x_j - n2_j                     (j-side, per k)
      row3_k[i]= r2 - |off_k|^2 - n2_i - 2off_k.x_i    (i-side, per k)
    h/row3 for all 9 cells come from two tiny f32 matmuls against coeff
    tables [3,9]/[4,9]; n2/ones rows are computed on device (ACT square +
    DVE add / memset), so only x,y ship on the j-side. The i-side basis
    [2x_i,2y_i,n2_i] (6KB) ships per core (the i-block differs per core and
    uniform SPMD code cannot address it dynamically).
  - Output ships bf16 [64,512] per core (512KB total vs 1MB f32).
  - The jitted executable is AOT-compiled via fast_dispatch_compile (C++
    fast-path dispatch, saves ~2ms/call of Python dispatch overhead).

Device pipeline per core (one batch, one 512-wide i-block):
  A-phase: AT[j,o] per (k, j-chunk) = matmul(dcoef chunk, wT k-block) -> bf16.
  T-phase: one K=4 f32 matmul per [128j x 512i] tile of t; ACT computes t^2,
  DVE w = max(t,0)*t^2 (bf16), PE accumulates out += AT.T @ w over 144 tiles
  into PSUM (two 64-row halves, summed at the end).
"""

import os
import sys

import numpy as np

for _p in ("/opt/trn_rl_repo", "/root/.axon_site/_ro/trn_rl_repo"):
    if os.path.isdir(_p) and _p not in sys.path:
        sys.path.append(_p)

import ml_dtypes  # noqa: E402

import concourse.bass as bass  # noqa: E402
import concourse.mybir as mybir  # noqa: E402
import concourse.tile as tile  # noqa: E402

# ---------------------------------------------------------------- constants
NDIM = 2
KSIZE = (3, 3)
DILATION = (0.05, 0.05)
RADIUS = 0.1
C_IN = 64
C_OUT = 64
B = 2
N = 2048
NCELLS = 9
R2 = RADIUS * RADIUS
KNORM = 315.0 / (64.0 * np.pi * RADIUS**9)

NCORES = 8
IBLK = 512           # i-columns per core
NCHUNK = N // 128    # 16 j-chunks of 128
M_TOT = NCELLS * NCHUNK  # 144 (k, j-chunk) tiles

F32 = mybir.dt.float32
BF16 = mybir.dt.bfloat16

# ---- per-batch blob layout (f32 word offsets; bf16 bit-packed) ----------
SZ_DAT = C_IN * N // 2              # dcoef bf16 [64,2048] -> 65536 words
SZ_POS = 2 * N                      # x row | y row f32    -> 4096
SZ_CH = 3 * NCELLS                  # h coeffs  [3,9]      -> 27
SZ_C4 = 4 * NCELLS                  # row3 coeffs [4,9]    -> 36
OFF_DAT = 0
OFF_POS = OFF_DAT + SZ_DAT
OFF_CH = OFF_POS + SZ_POS
OFF_C4 = OFF_CH + SZ_CH
BLOB_RAW = OFF_C4 + SZ_C4           # 69695
GSLICE = -(-BLOB_RAW // (4 * 16)) * 16  # 17424 words per core (x16 aligned)
BLOB = 4 * GSLICE                   # 69696

SZ_WT = C_IN * NCELLS * C_OUT // 2  # wT bf16 [64,576] -> 18432 words
WSLICE = SZ_WT // NCORES            # 2304 words per core

SZ_BU = 3 * IBLK                    # per-core [2x,2y,n2]_i f32 -> 1536
PACK = GSLICE + WSLICE + SZ_BU      # 21264 f32 words = 83.1 KiB per core
OFF_W = GSLICE
OFF_BU = GSLICE + WSLICE

_cache: dict = {}


# ------------------------------------------------- TileContext drain patch
# The walrus in this container rejects the Tile tail-drain when it carries
# more than ~2 sem waits ("Too many sync wait commands"). Split the waits
# over extra sync-engine NOPs, one wait each.
def _patch_tile_drain():
    if getattr(tile.TileContext, "_drain_patched", False):
        return
    import bass_rust
    from concourse.vector_clock import ScopedClock

    def _drain_and_barrier(self, tick_clock, wait_clock):
        drain_inst = self.nc.sync.drain()
        wait_clock.add_sem_waits(
            drain_inst.ins, ScopedClock({None: tick_clock.global_clock})
        )
        si = drain_inst.ins.sync_info
        waits = list(si.on_wait) if si is not None else []
        if len(waits) > 1:
            si.on_wait = waits[:1]
            drain_inst.ins.sync_info = si
            for w in waits[1:]:
                n = self.nc.sync.nop(nofuse=True, hint="drain_wait_split")
                n.ins.sync_info = bass_rust.SyncInfo(on_wait=[w], on_update=[])
        self.nc.all_engine_barrier()
        popped = self.nc._tile_sem_poison_stack.pop()
        assert popped is self._sem_poison
        self.nc.clear_and_free_semaphores(list(self.sems.allocated().values()))
        self.nc.all_engine_barrier()

    tile.TileContext._drain_and_barrier = _drain_and_barrier
    tile.TileContext._drain_patched = True


# --------------------------------------------- sync-wait legalization pass
# This walrus rejects instructions carrying more than ~1-2 sem waits. After
# Tile scheduling, move excess waits onto same-engine NoOps inserted right
# before the over-subscribed instruction (engines execute their stream in
# order, so semantics are identical).
_WAIT_LIMIT = 1


def _split_sync_waits(nc, limit=_WAIT_LIMIT):
    cnt = 0
    for f in nc.m.functions:
        for bb in f.blocks:
            changed = False
            out = []
            for inst in bb.instructions:
                si = inst.sync_info
                waits = list(si.on_wait) if si is not None else []
                if len(waits) > limit:
                    keep = waits[-limit:]
                    excess = waits[:-limit]
                    for j in range(0, len(excess), limit):
                        n = mybir.InstNoOp(
                            name=f"waitsplit_{cnt}",
                            engine=inst.engine,
                            ins=[],
                            outs=[],
                            sync_info=mybir.SyncInfo(
                                on_wait=excess[j : j + limit], on_update=[]
                            ),
                        )
                        cnt += 1
                        nc.register_instruction(n, overwrite=True)
                        out.append(n)
                    si.on_wait = keep
                    inst.sync_info = si
                    changed = True
                out.append(inst)
            if changed:
                bb.instructions = out
    return cnt


# ------------------------------------------------------------- device build
def _build_nc():
    _patch_tile_drain()

    nc = bass.Bass(num_devices=NCORES)
    pack_d = nc.declare_dram_parameter("pack", [1, PACK], F32, isOutput=False)
    out_d = nc.declare_dram_parameter("out", [C_OUT, IBLK + 4], mybir.dt.int8, isOutput=True)

    bu_v = pack_d[0:1, OFF_BU : OFF_BU + SZ_BU].rearrange(
        "a (p c) -> (a p) c", p=3
    )  # [3, 512] = [2x,2y,n2]_i

    from contextlib import ExitStack

    with tile.TileContext(nc) as tc, ExitStack() as ctx:
        dram = ctx.enter_context(tc.tile_pool(name="dram", bufs=1, space="DRAM"))
        const = ctx.enter_context(tc.tile_pool(name="const", bufs=1))
        qpool = ctx.enter_context(tc.tile_pool(name="q", bufs=4))
        wpool = ctx.enter_context(tc.tile_pool(name="w", bufs=4))
        apool = ctx.enter_context(tc.tile_pool(name="aps", bufs=2, space="PSUM"))
        tpool = ctx.enter_context(tc.tile_pool(name="t", bufs=2, space="PSUM"))
        opool = ctx.enter_context(tc.tile_pool(name="o", bufs=1, space="PSUM"))

        # ---- AllGather 1: per-batch blob (cores 0-3: b=0, cores 4-7: b=1)
        b_in = dram.tile([1, GSLICE], F32)
        flat = dram.tile([1, BLOB], F32)
        nc.gpsimd.dma_start(b_in[:], pack_d[0:1, 0:GSLICE])
        nc.gpsimd.collective_compute(
            "AllGather",
            mybir.AluOpType.bypass,
            replica_groups=[[0, 1, 2, 3], [4, 5, 6, 7]],
            ins=[b_in[:].opt()],
            outs=[flat[:].opt()],
        )
        # ---- AllGather 2: wT (batch-independent), 8-way
        w_in = dram.tile([1, WSLICE], F32)
        wgat = dram.tile([1, SZ_WT], F32)
        nc.gpsimd.dma_start(w_in[:], pack_d[0:1, OFF_W : OFF_W + WSLICE])
        nc.gpsimd.collective_compute(
            "AllGather",
            mybir.AluOpType.bypass,
            replica_groups=[[0, 1, 2, 3, 4, 5, 6, 7]],
            ins=[w_in[:].opt()],
            outs=[wgat[:].opt()],
        )

        dat_v = (
            flat[0:1, OFF_DAT : OFF_DAT + SZ_DAT]
            .bitcast(BF16)
            .rearrange("a (p c) -> (a p) c", p=C_IN)
        )  # [64, 2048] bf16
        xr_v = flat[0:1, OFF_POS : OFF_POS + N]            # [1, 2048]
        yr_v = flat[0:1, OFF_POS + N : OFF_POS + 2 * N]    # [1, 2048]
        ch_v = flat[0:1, OFF_CH : OFF_CH + SZ_CH].rearrange(
            "a (p c) -> (a p) c", p=3
        )  # [3, 9]
        c4_v = flat[0:1, OFF_C4 : OFF_C4 + SZ_C4].rearrange(
            "a (p c) -> (a p) c", p=4
        )  # [4, 9]
        wt_v = (
            wgat[0:1, :]
            .bitcast(BF16)
            .rearrange("a (p c) -> (a p) c", p=C_IN)
        )  # [64, 576] bf16

        datb = const.tile([C_IN, N], BF16)
        nc.sync.dma_start(datb[:], dat_v)
        wtb = const.tile([C_IN, NCELLS * C_OUT], BF16)
        nc.sync.dma_start(wtb[:], wt_v)

        # j-side basis rows (partition 0 each so ACT/DVE base-alignment holds)
        xr = const.tile([1, N], F32)
        nc.sync.dma_start(xr[:], xr_v)
        yr = const.tile([1, N], F32)
        nc.sync.dma_start(yr[:], yr_v)
        xsq = const.tile([1, N], F32)
        nc.scalar.square(xsq[:], xr[:])
        ysq = const.tile([1, N], F32)
        nc.scalar.square(ysq[:], yr[:])
        n2r = const.tile([1, N], F32)
        nc.vector.tensor_add(n2r[:], xsq[:], ysq[:])
        onesr = const.tile([1, N], F32)
        nc.vector.memset(onesr[:], 1.0)

        # matmul operands for the h / row3 coefficient matmuls
        bv_sb = const.tile([3, N], F32)     # [x, y, n2]_j
        nc.sync.dma_start(bv_sb[0:1, :], xr[:])
        nc.sync.dma_start(bv_sb[1:2, :], yr[:])
        nc.sync.dma_start(bv_sb[2:3, :], n2r[:])
        ch_sb = const.tile([3, NCELLS], F32)
        nc.sync.dma_start(ch_sb[:], ch_v)
        c4_sb = const.tile([4, NCELLS], F32)
        nc.sync.dma_start(c4_sb[:], c4_v)
        ub_sb = const.tile([4, IBLK], F32)  # [2x,2y,n2,1]_i basis
        nc.sync.dma_start(ub_sb[0:3, :], bu_v)
        nc.sync.dma_start(ub_sb[3:4, :], onesr[:, 0:IBLK])

        # V/U 4-row groups at 32-aligned partitions (PE tile-position rule).
        # Cell k lives at partition group g=k%4, column block cb=k//4.
        # V rows: [x_j, y_j, h_k[j], 1]   (x/y/1 k-independent, h per-k)
        # U rows: [2x_i, 2y_i, 1, row3_k[i]]
        vt = const.tile([128, 3 * N], F32)
        ut = const.tile([128, 3 * IBLK], F32)
        for k in range(NCELLS):
            g4, cb = 32 * (k % 4), k // 4
            nc.sync.dma_start(vt[g4 : g4 + 1, cb * N : (cb + 1) * N], xr[:])
            nc.sync.dma_start(vt[g4 + 1 : g4 + 2, cb * N : (cb + 1) * N], yr[:])
            nc.sync.dma_start(
                vt[g4 + 3 : g4 + 4, cb * N : (cb + 1) * N], onesr[:]
            )
            nc.sync.dma_start(
                ut[g4 : g4 + 2, cb * IBLK : (cb + 1) * IBLK], bu_v[0:2, :]
            )
            nc.sync.dma_start(
                ut[g4 + 2 : g4 + 3, cb * IBLK : (cb + 1) * IBLK],
                onesr[:, 0:IBLK],
            )

        # h[k, j] for all 9 cells: matmul against [x,y,n2] basis, 4 col chunks
        hstage = const.tile([NCELLS, N], F32)
        for cc in range(4):
            h_ps = opool.tile([NCELLS, 512], F32, tag="ops")
            nc.tensor.matmul(
                h_ps[:], ch_sb[:], bv_sb[:, cc * 512 : (cc + 1) * 512],
                start=True, stop=True,
            )
            nc.scalar.copy(hstage[:, cc * 512 : (cc + 1) * 512], h_ps[:])
        # row3[k, i] for the local i-block: one matmul
        r3stage = const.tile([NCELLS, IBLK], F32)
        r3_ps = opool.tile([NCELLS, IBLK], F32, tag="ops")
        nc.tensor.matmul(r3_ps[:], c4_sb[:], ub_sb[:], start=True, stop=True)
        nc.scalar.copy(r3stage[:], r3_ps[:])
        for k in range(NCELLS):
            g4, cb = 32 * (k % 4), k // 4
            nc.sync.dma_start(
                vt[g4 + 2 : g4 + 3, cb * N : (cb + 1) * N],
                hstage[k : k + 1, :],
            )
            nc.sync.dma_start(
                ut[g4 + 3 : g4 + 4, cb * IBLK : (cb + 1) * IBLK],
                r3stage[k : k + 1, :],
            )

        # A-phase: at[:, m*64:(m+1)*64] = AT chunk for m = k*16 + jc,
        # batched 8 matmuls per PSUM bank -> one ACT copy (f32->bf16) each.
        at = const.tile([128, M_TOT * C_OUT], BF16)
        for g in range(M_TOT // 8):
            a_ps = apool.tile([128, 512], F32)
            for r in range(8):
                m = g * 8 + r
                k, jc = divmod(m, NCHUNK)
                nc.tensor.matmul(
                    a_ps[:, r * 64 : (r + 1) * 64],
                    datb[:, jc * 128 : (jc + 1) * 128],
                    wtb[:, k * 64 : (k + 1) * 64],
                    start=True,
                    stop=True,
                )
            nc.scalar.copy(at[:, g * 512 : (g + 1) * 512], a_ps[:])

        # T-phase + main accumulation, software-pipelined: t-matmuls and
        # ACT/DVE cube for pair p run while PE accumulates mains of p-2.
        out_ps = opool.tile([128, IBLK], F32, tag="ops")
        pend = []  # (m, w_tile, half) awaiting the main matmul
        for gp in range(M_TOT // 2):
            t_ps = tpool.tile([128, 2 * IBLK], F32)
            for r in range(2):
                m = gp * 2 + r
                k, jc = divmod(m, NCHUNK)
                g4, cb = 32 * (k % 4), k // 4
                nc.tensor.matmul(
                    t_ps[:, r * IBLK : (r + 1) * IBLK],
                    vt[g4 : g4 + 4, cb * N + jc * 128 : cb * N + jc * 128 + 128],
                    ut[g4 : g4 + 4, cb * IBLK : (cb + 1) * IBLK],
                    start=True,
                    stop=True,
                    tile_position=(g4, 0),
                )
            q_t = qpool.tile([128, 2 * IBLK], BF16)
            nc.scalar.square(q_t[:], t_ps[:])
            w_t = wpool.tile([128, 2 * IBLK], BF16)
            nc.vector.scalar_tensor_tensor(
                w_t[:], t_ps[:], 0.0, q_t[:],
                op0=mybir.AluOpType.max, op1=mybir.AluOpType.mult,
            )
            pend.append((gp * 2, w_t, 0))
            pend.append((gp * 2 + 1, w_t, 1))
            while len(pend) > 4 or (gp == M_TOT // 2 - 1 and pend):
                m, w_tile, r = pend.pop(0)
                par = (m % 2) * C_OUT
                nc.tensor.matmul(
                    out_ps[par : par + C_OUT, :],
                    at[:, m * C_OUT : (m + 1) * C_OUT],
                    w_tile[:, r * IBLK : (r + 1) * IBLK],
                    start=(m < 2),
                    stop=(m >= M_TOT - 2),
                    skip_group_check=True,
                    tile_position=(0, par),
                )

        tmp_sb = const.tile([C_OUT, IBLK], F32)
        nc.scalar.copy(tmp_sb[:], out_ps[0:C_OUT, :])
        osum = const.tile([C_OUT, IBLK], F32)
        nc.vector.tensor_add(osum[:], tmp_sb[:], out_ps[C_OUT:, :])
        # int8 output: per-row abs-max scale, q = round(osum * 127/rowmax)
        rowmax = const.tile([C_OUT, 1], F32)
        nc.vector.tensor_reduce(
            rowmax[:], osum[:], mybir.AxisListType.X, mybir.AluOpType.max,
            apply_absolute_value=True,
        )
        nc.vector.tensor_scalar_max(rowmax[:], rowmax[:], 1e-20)
        scale = const.tile([C_OUT, 1], F32)
        nc.vector.reciprocal(scale[:], rowmax[:])
        nc.vector.tensor_scalar_mul(scale[:], scale[:], 127.0)
        q8 = const.tile([C_OUT, IBLK], mybir.dt.int8)
        nc.vector.tensor_scalar(
            q8[:], osum[:], scale[:], None, op0=mybir.AluOpType.mult
        )
        nc.sync.dma_start(out_d[:, 0:IBLK], q8[:])
        nc.sync.dma_start(out_d[:, IBLK : IBLK + 4].bitcast(F32), rowmax[:])
    _split_sync_waits(nc)
    return nc


# ------------------------------------------------------------- cached runner
def _get_runner():
    """Build (once) the jitted SPMD executable: pack [8, PACK] -> out bf16.

    Same bass_exec/PJRT machinery as bass_utils.run_bass_kernel_spmd under
    axon, but the executable is cached so repeat launches skip retracing,
    and AOT-compiled under fast_dispatch_compile (C++ fast-path dispatch).
    """
    if "runner" in _cache:
        return _cache["runner"]

    import jax
    from jax.sharding import Mesh, PartitionSpec

    from jax.experimental.shard_map import shard_map
    from concourse import bass2jax

    bass2jax.install_neuronx_cc_hook()

    nc = _build_nc()

    partition_name = (
        nc.partition_id_tensor.name if nc.partition_id_tensor else None
    )
    in_names, out_names, out_avals = [], [], []
    for alloc in nc.m.functions[0].allocations:
        if not isinstance(alloc, mybir.MemoryLocationSet):
            continue
        name = alloc.memorylocations[0].name
        if alloc.kind == "ExternalInput":
            if name != partition_name:
                in_names.append(name)
        elif alloc.kind == "ExternalOutput":
            out_avals.append(
                jax.core.ShapedArray(
                    tuple(alloc.tensor_shape), mybir.dt.np(alloc.dtype)
                )
            )
            out_names.append(name)
    assert in_names == ["pack"] and out_names == ["out"], (in_names, out_names)
    # "out" is deliberately NOT an operand: the kernel writes every element,
    # so no pre-zeroed donated buffer needs to ship through the tunnel.
    in_names_full = in_names + ([partition_name] if partition_name else [])

    def _body(pack):
        operands = [pack]
        if partition_name is not None:
            operands.append(bass2jax.partition_id_tensor())
        outs = bass2jax._bass_exec_p.bind(
            *operands,
            out_avals=tuple(out_avals),
            in_names=tuple(in_names_full),
            out_names=tuple(out_names),
            lowering_input_output_aliases=(),
            sim_require_finite=True,
            sim_require_nnan=True,
            nc=nc,
        )
        return outs[0]

    devices = jax.devices()[:NCORES]
    assert len(devices) == NCORES, f"need {NCORES} devices, got {len(devices)}"
    mesh = Mesh(np.asarray(devices), ("core",))

    def _make_jit():
        return jax.jit(
            shard_map(
                _body,
                mesh=mesh,
                in_specs=(PartitionSpec("core"),),
                out_specs=PartitionSpec("core"),
                check_rep=False,
            )
        )

    try:
        fn = bass2jax.fast_dispatch_compile(
            lambda: _make_jit()
            .lower(jax.ShapeDtypeStruct((NCORES, PACK), np.float32))
            .compile()
        )
    except Exception:
        fn = _make_jit()
    _cache["runner"] = fn
    return fn


# ------------------------------------------------------------ host wrapper
def _offsets():
    axes = [
        (np.arange(kk) - (kk - 1) / 2.0) * d for kk, d in zip(KSIZE, DILATION)
    ]
    grids = np.meshgrid(*axes, indexing="ij")
    return np.stack([g.reshape(-1) for g in grids], axis=-1).astype(np.float32)


def _prepare_pack(locs, data, density, weight):
    locs = np.asarray(locs, np.float32)
    data = np.asarray(data, np.float32)
    density = np.asarray(density, np.float32)
    weight = np.asarray(weight, np.float32)

    pos = locs[..., :NDIM]                        # [B,N,2]
    invmass = locs[..., NDIM]                     # [B,N]
    coef = 1.0 / (invmass * density)              # [B,N]
    dco = (data * coef[:, None, :]).astype(ml_dtypes.bfloat16)  # [B,64,N]
    # wT[c, k*64+o] = weight[o,c,k] * knorm
    wt = np.ascontiguousarray(weight.transpose(1, 2, 0) * np.float32(KNORM))
    wflat = wt.astype(ml_dtypes.bfloat16).reshape(C_IN, -1).view(np.float32)
    wflat = np.ascontiguousarray(wflat).ravel()   # [18432]
    offs = _offsets()                             # [9,2]

    ch = np.empty((3, NCELLS), np.float32)        # h = 2off.x_j - n2_j
    ch[0] = 2.0 * offs[:, 0]
    ch[1] = 2.0 * offs[:, 1]
    ch[2] = -1.0
    c4 = np.empty((4, NCELLS), np.float32)        # row3 on [2x,2y,n2,1]_i
    c4[0] = -offs[:, 0]
    c4[1] = -offs[:, 1]
    c4[2] = -1.0
    c4[3] = R2 - (offs**2).sum(1)

    pack = np.empty((NCORES, PACK), np.float32)
    blob = np.empty(BLOB, np.float32)
    blob[BLOB_RAW:] = 0.0
    for b in range(B):
        x, y = pos[b, :, 0], pos[b, :, 1]
        n2 = x * x + y * y
        blob[OFF_DAT : OFF_DAT + SZ_DAT] = (
            np.ascontiguousarray(dco[b]).view(np.float32).ravel()
        )
        blob[OFF_POS : OFF_POS + N] = x
        blob[OFF_POS + N : OFF_POS + 2 * N] = y
        blob[OFF_CH : OFF_CH + SZ_CH] = ch.ravel()
        blob[OFF_C4 : OFF_C4 + SZ_C4] = c4.ravel()
        bu = np.stack([2.0 * x, 2.0 * y, n2])     # [3, 2048]
        for q in range(4):
            c = b * 4 + q
            p = pack[c]
            p[0:GSLICE] = blob[q * GSLICE : (q + 1) * GSLICE]
            p[OFF_W : OFF_W + WSLICE] = wflat[c * WSLICE : (c + 1) * WSLICE]
            p[OFF_BU:] = bu[:, q * IBLK : (q + 1) * IBLK].ravel()
    return pack


def _launch(pack):
    fn = _get_runner()
    res = np.asarray(fn(pack))                    # [8*64, 516] int8
    res = res.reshape(NCORES, C_OUT, IBLK + 4)
    q = res[:, :, :IBLK].astype(np.float32)
    rowmax = np.ascontiguousarray(res[:, :, IBLK:]).view(np.float32)  # [8,64,1]
    return q * (rowmax / np.float32(127.0))


def _unpack_out(res, bias):
    out = np.empty((B, C_OUT, N), np.float32)
    for c in range(NCORES):
        b, q = divmod(c, 4)
        out[b][:, q * IBLK : (q + 1) * IBLK] = res[c]
    out += np.asarray(bias, np.float32)[None, :, None]
    return out


def kernel(locs, data, density, weight, bias):
    pack = _prepare_pack(locs, data, density, weight)
    return _unpack_out(_launch(pack), bias)


# -------------------------------------------------------------- benchmarking
def time_kernel(locs, data, density, weight, bias, iters=12):
    """Return (best_wall_s, per_call_s_list) for device launches.

    Host-side input prep runs once outside the loop; each timed iteration
    covers shipping the packed inputs to the 8 cores, executing, fetching,
    and unsharding the output.
    """
    import time

    kernel(locs, data, density, weight, bias)  # warm (compile)
    pack = _prepare_pack(locs, data, density, weight)
    times = []
    for _ in range(iters):
        t0 = time.perf_counter()
        _unpack_out(_launch(pack), bias)
        times.append(time.perf_counter() - t0)
    return min(times), times


# revision 7
# speedup vs baseline: 1.3233x; 1.2969x over previous
"""ConvSP (SPH smoothing-kernel convolution) Trainium2 Bass kernel, v4.

Math (per batch b):
  out[o,i] = bias[o] + sum_k sum_j A_k[o,j] * relu(r2 - |x_i - x_j + off_k|^2)^3
  A_k[o,j] = knorm * sum_c weight[o,c,k] * dcoef[c,j],  dcoef = data / (invmass*density)

Wall-clock is dominated by the axon tunnel: ~40-80ms RTT (time-varying) plus
~75MB/s up / ~45MB/s down. v1 shipped 688KB/core (5.5MB total, payloads
replicated 4-8x). v3 ships each core ~83KB of UNIQUE bytes (666KB total) and
reconstructs everything else on device:

  - dcoef (bf16) + raw positions + tiny coeff tables form a per-batch blob,
    split 4 ways; an on-device HBM AllGather over replica groups
    [[0-3],[4-7]] rebuilds it on every core (NeuronLink, ~us).
  - wT*knorm (bf16, batch-independent) is split 8 ways and AllGathered
    across all cores.
  - t[j,i] = r2 - |x_i - x_j + off_k|^2 is a rank-4 bilinear form
      t = x_j*(2x_i) + y_j*(2y_i) + h_k[j]*1 + 1*row3_k[i]
      h_k[j]   = 2off_k.x_j - n2_j                     (j-side, per k)
      row3_k[i]= r2 - |off_k|^2 - n2_i - 2off_k.x_i    (i-side, per k)
    h/row3 for all 9 cells come from two tiny f32 matmuls against coeff
    tables [3,9]/[4,9]; n2/ones rows are computed on device (ACT square +
    DVE add / memset), so only x,y ship on the j-side. The i-side basis
    [2x_i,2y_i,n2_i] (6KB) ships per core (the i-block differs per core and
    uniform SPMD code cannot address it dynamically).
  - Output ships int8 [64,512] + per-row f32 abs-max scales (264KB total vs
    1MB f32): on-device DVE abs-max reduce + reciprocal + quantize, host
    dequant. Measured rel err 7.4-7.6e-3 across seeds (gate 2e-2).
  - The jitted executable is AOT-compiled via fast_dispatch_compile (C++
    fast-path dispatch, saves ~2ms/call of Python dispatch overhead).

Device pipeline per core (one batch, one 512-wide i-block):
  A-phase: AT[j,o] per (k, j-chunk) = matmul(dcoef chunk, wT k-block) -> bf16.
  T-phase: one K=4 f32 matmul per [128j x 512i] tile of t; ACT computes t^2,
  DVE w = max(t,0)*t^2 (bf16), PE accumulates out += AT.T @ w over 144 tiles
  into PSUM (two 64-row halves, summed at the end).
"""

import os
import sys

import numpy as np

for _p in ("/opt/trn_rl_repo", "/root/.axon_site/_ro/trn_rl_repo"):
    if os.path.isdir(_p) and _p not in sys.path:
        sys.path.append(_p)

import ml_dtypes  # noqa: E402

import concourse.bass as bass  # noqa: E402
import concourse.mybir as mybir  # noqa: E402
import concourse.tile as tile  # noqa: E402

# ---------------------------------------------------------------- constants
NDIM = 2
KSIZE = (3, 3)
DILATION = (0.05, 0.05)
RADIUS = 0.1
C_IN = 64
C_OUT = 64
B = 2
N = 2048
NCELLS = 9
R2 = RADIUS * RADIUS
KNORM = 315.0 / (64.0 * np.pi * RADIUS**9)

NCORES = 8
IBLK = 512           # i-columns per core
NCHUNK = N // 128    # 16 j-chunks of 128
M_TOT = NCELLS * NCHUNK  # 144 (k, j-chunk) tiles

F32 = mybir.dt.float32
BF16 = mybir.dt.bfloat16

# ---- per-batch blob layout (f32 word offsets; bf16 bit-packed) ----------
SZ_DAT = C_IN * N // 2              # dcoef bf16 [64,2048] -> 65536 words
SZ_POS = 2 * N                      # x row | y row f32    -> 4096
SZ_CH = 3 * NCELLS                  # h coeffs  [3,9]      -> 27
SZ_C4 = 4 * NCELLS                  # row3 coeffs [4,9]    -> 36
OFF_DAT = 0
OFF_POS = OFF_DAT + SZ_DAT
OFF_CH = OFF_POS + SZ_POS
OFF_C4 = OFF_CH + SZ_CH
BLOB_RAW = OFF_C4 + SZ_C4           # 69695
GSLICE = -(-BLOB_RAW // (4 * 16)) * 16  # 17424 words per core (x16 aligned)
BLOB = 4 * GSLICE                   # 69696

SZ_WT = C_IN * NCELLS * C_OUT // 2  # wT bf16 [64,576] -> 18432 words
WSLICE = SZ_WT // NCORES            # 2304 words per core

SZ_BU = 3 * IBLK                    # per-core [2x,2y,n2]_i f32 -> 1536
PACK = GSLICE + WSLICE + SZ_BU      # 21264 f32 words = 83.1 KiB per core
OFF_W = GSLICE
OFF_BU = GSLICE + WSLICE

_cache: dict = {}


# ------------------------------------------------- TileContext drain patch
# The walrus in this container rejects the Tile tail-drain when it carries
# more than ~2 sem waits ("Too many sync wait commands"). Split the waits
# over extra sync-engine NOPs, one wait each.
def _patch_tile_drain():
    if getattr(tile.TileContext, "_drain_patched", False):
        return
    import bass_rust
    from concourse.vector_clock import ScopedClock

    def _drain_and_barrier(self, tick_clock, wait_clock):
        drain_inst = self.nc.sync.drain()
        wait_clock.add_sem_waits(
            drain_inst.ins, ScopedClock({None: tick_clock.global_clock})
        )
        si = drain_inst.ins.sync_info
        waits = list(si.on_wait) if si is not None else []
        if len(waits) > 1:
            si.on_wait = waits[:1]
            drain_inst.ins.sync_info = si
            for w in waits[1:]:
                n = self.nc.sync.nop(nofuse=True, hint="drain_wait_split")
                n.ins.sync_info = bass_rust.SyncInfo(on_wait=[w], on_update=[])
        self.nc.all_engine_barrier()
        popped = self.nc._tile_sem_poison_stack.pop()
        assert popped is self._sem_poison
        self.nc.clear_and_free_semaphores(list(self.sems.allocated().values()))
        self.nc.all_engine_barrier()

    tile.TileContext._drain_and_barrier = _drain_and_barrier
    tile.TileContext._drain_patched = True


# --------------------------------------------- sync-wait legalization pass
# This walrus rejects instructions carrying more than ~1-2 sem waits. After
# Tile scheduling, move excess waits onto same-engine NoOps inserted right
# before the over-subscribed instruction (engines execute their stream in
# order, so semantics are identical).
_WAIT_LIMIT = 1


def _split_sync_waits(nc, limit=_WAIT_LIMIT):
    cnt = 0
    for f in nc.m.functions:
        for bb in f.blocks:
            changed = False
            out = []
            for inst in bb.instructions:
                si = inst.sync_info
                waits = list(si.on_wait) if si is not None else []
                if len(waits) > limit:
                    keep = waits[-limit:]
                    excess = waits[:-limit]
                    for j in range(0, len(excess), limit):
                        n = mybir.InstNoOp(
                            name=f"waitsplit_{cnt}",
                            engine=inst.engine,
                            ins=[],
                            outs=[],
                            sync_info=mybir.SyncInfo(
                                on_wait=excess[j : j + limit], on_update=[]
                            ),
                        )
                        cnt += 1
                        nc.register_instruction(n, overwrite=True)
                        out.append(n)
                    si.on_wait = keep
                    inst.sync_info = si
                    changed = True
                out.append(inst)
            if changed:
                bb.instructions = out
    return cnt


# ------------------------------------------------------------- device build
def _build_nc():
    _patch_tile_drain()

    nc = bass.Bass(num_devices=NCORES)
    pack_d = nc.declare_dram_parameter("pack", [1, PACK], F32, isOutput=False)
    out_d = nc.declare_dram_parameter("out", [C_OUT, IBLK + 4], mybir.dt.int8, isOutput=True)

    bu_v = pack_d[0:1, OFF_BU : OFF_BU + SZ_BU].rearrange(
        "a (p c) -> (a p) c", p=3
    )  # [3, 512] = [2x,2y,n2]_i

    from contextlib import ExitStack

    with tile.TileContext(nc) as tc, ExitStack() as ctx:
        dram = ctx.enter_context(tc.tile_pool(name="dram", bufs=1, space="DRAM"))
        const = ctx.enter_context(tc.tile_pool(name="const", bufs=1))
        qpool = ctx.enter_context(tc.tile_pool(name="q", bufs=4))
        wpool = ctx.enter_context(tc.tile_pool(name="w", bufs=4))
        apool = ctx.enter_context(tc.tile_pool(name="aps", bufs=2, space="PSUM"))
        tpool = ctx.enter_context(tc.tile_pool(name="t", bufs=2, space="PSUM"))
        opool = ctx.enter_context(tc.tile_pool(name="o", bufs=1, space="PSUM"))

        # ---- AllGather 1: per-batch blob (cores 0-3: b=0, cores 4-7: b=1)
        b_in = dram.tile([1, GSLICE], F32)
        flat = dram.tile([1, BLOB], F32)
        nc.gpsimd.dma_start(b_in[:], pack_d[0:1, 0:GSLICE])
        nc.gpsimd.collective_compute(
            "AllGather",
            mybir.AluOpType.bypass,
            replica_groups=[[0, 1, 2, 3], [4, 5, 6, 7]],
            ins=[b_in[:].opt()],
            outs=[flat[:].opt()],
        )
        # ---- AllGather 2: wT (batch-independent), 8-way
        w_in = dram.tile([1, WSLICE], F32)
        wgat = dram.tile([1, SZ_WT], F32)
        nc.gpsimd.dma_start(w_in[:], pack_d[0:1, OFF_W : OFF_W + WSLICE])
        nc.gpsimd.collective_compute(
            "AllGather",
            mybir.AluOpType.bypass,
            replica_groups=[[0, 1, 2, 3, 4, 5, 6, 7]],
            ins=[w_in[:].opt()],
            outs=[wgat[:].opt()],
        )

        dat_v = (
            flat[0:1, OFF_DAT : OFF_DAT + SZ_DAT]
            .bitcast(BF16)
            .rearrange("a (p c) -> (a p) c", p=C_IN)
        )  # [64, 2048] bf16
        xr_v = flat[0:1, OFF_POS : OFF_POS + N]            # [1, 2048]
        yr_v = flat[0:1, OFF_POS + N : OFF_POS + 2 * N]    # [1, 2048]
        ch_v = flat[0:1, OFF_CH : OFF_CH + SZ_CH].rearrange(
            "a (p c) -> (a p) c", p=3
        )  # [3, 9]
        c4_v = flat[0:1, OFF_C4 : OFF_C4 + SZ_C4].rearrange(
            "a (p c) -> (a p) c", p=4
        )  # [4, 9]
        wt_v = (
            wgat[0:1, :]
            .bitcast(BF16)
            .rearrange("a (p c) -> (a p) c", p=C_IN)
        )  # [64, 576] bf16

        datb = const.tile([C_IN, N], BF16)
        nc.sync.dma_start(datb[:], dat_v)
        wtb = const.tile([C_IN, NCELLS * C_OUT], BF16)
        nc.sync.dma_start(wtb[:], wt_v)

        # j-side basis rows (partition 0 each so ACT/DVE base-alignment holds)
        xr = const.tile([1, N], F32)
        nc.sync.dma_start(xr[:], xr_v)
        yr = const.tile([1, N], F32)
        nc.sync.dma_start(yr[:], yr_v)
        xsq = const.tile([1, N], F32)
        nc.scalar.square(xsq[:], xr[:])
        ysq = const.tile([1, N], F32)
        nc.scalar.square(ysq[:], yr[:])
        n2r = const.tile([1, N], F32)
        nc.vector.tensor_add(n2r[:], xsq[:], ysq[:])
        onesr = const.tile([1, N], F32)
        nc.vector.memset(onesr[:], 1.0)

        # matmul operands for the h / row3 coefficient matmuls
        bv_sb = const.tile([3, N], F32)     # [x, y, n2]_j
        nc.sync.dma_start(bv_sb[0:1, :], xr[:])
        nc.sync.dma_start(bv_sb[1:2, :], yr[:])
        nc.sync.dma_start(bv_sb[2:3, :], n2r[:])
        ch_sb = const.tile([3, NCELLS], F32)
        nc.sync.dma_start(ch_sb[:], ch_v)
        c4_sb = const.tile([4, NCELLS], F32)
        nc.sync.dma_start(c4_sb[:], c4_v)
        ub_sb = const.tile([4, IBLK], F32)  # [2x,2y,n2,1]_i basis
        nc.sync.dma_start(ub_sb[0:3, :], bu_v)
        nc.sync.dma_start(ub_sb[3:4, :], onesr[:, 0:IBLK])

        # V/U 4-row groups at 32-aligned partitions (PE tile-position rule).
        # Cell k lives at partition group g=k%4, column block cb=k//4.
        # V rows: [x_j, y_j, h_k[j], 1]   (x/y/1 k-independent, h per-k)
        # U rows: [2x_i, 2y_i, 1, row3_k[i]]
        vt = const.tile([128, 3 * N], F32)
        ut = const.tile([128, 3 * IBLK], F32)
        for k in range(NCELLS):
            g4, cb = 32 * (k % 4), k // 4
            nc.sync.dma_start(vt[g4 : g4 + 1, cb * N : (cb + 1) * N], xr[:])
            nc.sync.dma_start(vt[g4 + 1 : g4 + 2, cb * N : (cb + 1) * N], yr[:])
            nc.sync.dma_start(
                vt[g4 + 3 : g4 + 4, cb * N : (cb + 1) * N], onesr[:]
            )
            nc.sync.dma_start(
                ut[g4 : g4 + 2, cb * IBLK : (cb + 1) * IBLK], bu_v[0:2, :]
            )
            nc.sync.dma_start(
                ut[g4 + 2 : g4 + 3, cb * IBLK : (cb + 1) * IBLK],
                onesr[:, 0:IBLK],
            )

        # h[k, j] for all 9 cells: matmul against [x,y,n2] basis, 4 col chunks
        hstage = const.tile([NCELLS, N], F32)
        for cc in range(4):
            h_ps = opool.tile([NCELLS, 512], F32, tag="ops")
            nc.tensor.matmul(
                h_ps[:], ch_sb[:], bv_sb[:, cc * 512 : (cc + 1) * 512],
                start=True, stop=True,
            )
            nc.scalar.copy(hstage[:, cc * 512 : (cc + 1) * 512], h_ps[:])
        # row3[k, i] for the local i-block: one matmul
        r3stage = const.tile([NCELLS, IBLK], F32)
        r3_ps = opool.tile([NCELLS, IBLK], F32, tag="ops")
        nc.tensor.matmul(r3_ps[:], c4_sb[:], ub_sb[:], start=True, stop=True)
        nc.scalar.copy(r3stage[:], r3_ps[:])
        for k in range(NCELLS):
            g4, cb = 32 * (k % 4), k // 4
            nc.sync.dma_start(
                vt[g4 + 2 : g4 + 3, cb * N : (cb + 1) * N],
                hstage[k : k + 1, :],
            )
            nc.sync.dma_start(
                ut[g4 + 3 : g4 + 4, cb * IBLK : (cb + 1) * IBLK],
                r3stage[k : k + 1, :],
            )

        # A-phase: at[:, m*64:(m+1)*64] = AT chunk for m = k*16 + jc,
        # batched 8 matmuls per PSUM bank -> one ACT copy (f32->bf16) each.
        at = const.tile([128, M_TOT * C_OUT], BF16)
        for g in range(M_TOT // 8):
            a_ps = apool.tile([128, 512], F32)
            for r in range(8):
                m = g * 8 + r
                k, jc = divmod(m, NCHUNK)
                nc.tensor.matmul(
                    a_ps[:, r * 64 : (r + 1) * 64],
                    datb[:, jc * 128 : (jc + 1) * 128],
                    wtb[:, k * 64 : (k + 1) * 64],
                    start=True,
                    stop=True,
                )
            nc.scalar.copy(at[:, g * 512 : (g + 1) * 512], a_ps[:])

        # T-phase + main accumulation, software-pipelined: t-matmuls and
        # ACT/DVE cube for pair p run while PE accumulates mains of p-2.
        out_ps = opool.tile([128, IBLK], F32, tag="ops")
        pend = []  # (m, w_tile, half) awaiting the main matmul
        for gp in range(M_TOT // 2):
            t_ps = tpool.tile([128, 2 * IBLK], F32)
            for r in range(2):
                m = gp * 2 + r
                k, jc = divmod(m, NCHUNK)
                g4, cb = 32 * (k % 4), k // 4
                nc.tensor.matmul(
                    t_ps[:, r * IBLK : (r + 1) * IBLK],
                    vt[g4 : g4 + 4, cb * N + jc * 128 : cb * N + jc * 128 + 128],
                    ut[g4 : g4 + 4, cb * IBLK : (cb + 1) * IBLK],
                    start=True,
                    stop=True,
                    tile_position=(g4, 0),
                )
            q_t = qpool.tile([128, 2 * IBLK], BF16)
            nc.scalar.square(q_t[:], t_ps[:])
            w_t = wpool.tile([128, 2 * IBLK], BF16)
            nc.vector.scalar_tensor_tensor(
                w_t[:], t_ps[:], 0.0, q_t[:],
                op0=mybir.AluOpType.max, op1=mybir.AluOpType.mult,
            )
            pend.append((gp * 2, w_t, 0))
            pend.append((gp * 2 + 1, w_t, 1))
            while len(pend) > 4 or (gp == M_TOT // 2 - 1 and pend):
                m, w_tile, r = pend.pop(0)
                par = (m % 2) * C_OUT
                nc.tensor.matmul(
                    out_ps[par : par + C_OUT, :],
                    at[:, m * C_OUT : (m + 1) * C_OUT],
                    w_tile[:, r * IBLK : (r + 1) * IBLK],
                    start=(m < 2),
                    stop=(m >= M_TOT - 2),
                    skip_group_check=True,
                    tile_position=(0, par),
                )

        tmp_sb = const.tile([C_OUT, IBLK], F32)
        nc.scalar.copy(tmp_sb[:], out_ps[0:C_OUT, :])
        osum = const.tile([C_OUT, IBLK], F32)
        nc.vector.tensor_add(osum[:], tmp_sb[:], out_ps[C_OUT:, :])
        # int8 output: per-row abs-max scale, q = round(osum * 127/rowmax)
        rowmax = const.tile([C_OUT, 1], F32)
        nc.vector.tensor_reduce(
            rowmax[:], osum[:], mybir.AxisListType.X, mybir.AluOpType.max,
            apply_absolute_value=True,
        )
        nc.vector.tensor_scalar_max(rowmax[:], rowmax[:], 1e-20)
        scale = const.tile([C_OUT, 1], F32)
        nc.vector.reciprocal(scale[:], rowmax[:])
        nc.vector.tensor_scalar_mul(scale[:], scale[:], 127.0)
        q8 = const.tile([C_OUT, IBLK], mybir.dt.int8)
        nc.vector.tensor_scalar(
            q8[:], osum[:], scale[:], None, op0=mybir.AluOpType.mult
        )
        nc.sync.dma_start(out_d[:, 0:IBLK], q8[:])
        nc.sync.dma_start(out_d[:, IBLK : IBLK + 4].bitcast(F32), rowmax[:])
    _split_sync_waits(nc)
    return nc


# ------------------------------------------------------------- cached runner
def _get_runner():
    """Build (once) the jitted SPMD executable: pack [8, PACK] -> out bf16.

    Same bass_exec/PJRT machinery as bass_utils.run_bass_kernel_spmd under
    axon, but the executable is cached so repeat launches skip retracing,
    and AOT-compiled under fast_dispatch_compile (C++ fast-path dispatch).
    """
    if "runner" in _cache:
        return _cache["runner"]

    import jax
    from jax.sharding import Mesh, PartitionSpec

    from jax.experimental.shard_map import shard_map
    from concourse import bass2jax

    bass2jax.install_neuronx_cc_hook()

    nc = _build_nc()

    partition_name = (
        nc.partition_id_tensor.name if nc.partition_id_tensor else None
    )
    in_names, out_names, out_avals = [], [], []
    for alloc in nc.m.functions[0].allocations:
        if not isinstance(alloc, mybir.MemoryLocationSet):
            continue
        name = alloc.memorylocations[0].name
        if alloc.kind == "ExternalInput":
            if name != partition_name:
                in_names.append(name)
        elif alloc.kind == "ExternalOutput":
            out_avals.append(
                jax.core.ShapedArray(
                    tuple(alloc.tensor_shape), mybir.dt.np(alloc.dtype)
                )
            )
            out_names.append(name)
    assert in_names == ["pack"] and out_names == ["out"], (in_names, out_names)
    # "out" is deliberately NOT an operand: the kernel writes every element,
    # so no pre-zeroed donated buffer needs to ship through the tunnel.
    in_names_full = in_names + ([partition_name] if partition_name else [])

    def _body(pack):
        operands = [pack]
        if partition_name is not None:
            operands.append(bass2jax.partition_id_tensor())
        outs = bass2jax._bass_exec_p.bind(
            *operands,
            out_avals=tuple(out_avals),
            in_names=tuple(in_names_full),
            out_names=tuple(out_names),
            lowering_input_output_aliases=(),
            sim_require_finite=True,
            sim_require_nnan=True,
            nc=nc,
        )
        return outs[0]

    devices = jax.devices()[:NCORES]
    assert len(devices) == NCORES, f"need {NCORES} devices, got {len(devices)}"
    mesh = Mesh(np.asarray(devices), ("core",))

    def _make_jit():
        return jax.jit(
            shard_map(
                _body,
                mesh=mesh,
                in_specs=(PartitionSpec("core"),),
                out_specs=PartitionSpec("core"),
                check_rep=False,
            )
        )

    try:
        fn = bass2jax.fast_dispatch_compile(
            lambda: _make_jit()
            .lower(jax.ShapeDtypeStruct((NCORES, PACK), np.float32))
            .compile()
        )
    except Exception:
        fn = _make_jit()
    _cache["runner"] = fn
    return fn


# ------------------------------------------------------------ host wrapper
def _offsets():
    axes = [
        (np.arange(kk) - (kk - 1) / 2.0) * d for kk, d in zip(KSIZE, DILATION)
    ]
    grids = np.meshgrid(*axes, indexing="ij")
    return np.stack([g.reshape(-1) for g in grids], axis=-1).astype(np.float32)


def _prepare_pack(locs, data, density, weight):
    locs = np.asarray(locs, np.float32)
    data = np.asarray(data, np.float32)
    density = np.asarray(density, np.float32)
    weight = np.asarray(weight, np.float32)

    pos = locs[..., :NDIM]                        # [B,N,2]
    invmass = locs[..., NDIM]                     # [B,N]
    coef = 1.0 / (invmass * density)              # [B,N]
    dco = (data * coef[:, None, :]).astype(ml_dtypes.bfloat16)  # [B,64,N]
    # wT[c, k*64+o] = weight[o,c,k] * knorm
    wt = np.ascontiguousarray(weight.transpose(1, 2, 0) * np.float32(KNORM))
    wflat = wt.astype(ml_dtypes.bfloat16).reshape(C_IN, -1).view(np.float32)
    wflat = np.ascontiguousarray(wflat).ravel()   # [18432]
    offs = _offsets()                             # [9,2]

    ch = np.empty((3, NCELLS), np.float32)        # h = 2off.x_j - n2_j
    ch[0] = 2.0 * offs[:, 0]
    ch[1] = 2.0 * offs[:, 1]
    ch[2] = -1.0
    c4 = np.empty((4, NCELLS), np.float32)        # row3 on [2x,2y,n2,1]_i
    c4[0] = -offs[:, 0]
    c4[1] = -offs[:, 1]
    c4[2] = -1.0
    c4[3] = R2 - (offs**2).sum(1)

    pack = np.empty((NCORES, PACK), np.float32)
    blob = np.empty(BLOB, np.float32)
    blob[BLOB_RAW:] = 0.0
    for b in range(B):
        x, y = pos[b, :, 0], pos[b, :, 1]
        n2 = x * x + y * y
        blob[OFF_DAT : OFF_DAT + SZ_DAT] = (
            np.ascontiguousarray(dco[b]).view(np.float32).ravel()
        )
        blob[OFF_POS : OFF_POS + N] = x
        blob[OFF_POS + N : OFF_POS + 2 * N] = y
        blob[OFF_CH : OFF_CH + SZ_CH] = ch.ravel()
        blob[OFF_C4 : OFF_C4 + SZ_C4] = c4.ravel()
        bu = np.stack([2.0 * x, 2.0 * y, n2])     # [3, 2048]
        for q in range(4):
            c = b * 4 + q
            p = pack[c]
            p[0:GSLICE] = blob[q * GSLICE : (q + 1) * GSLICE]
            p[OFF_W : OFF_W + WSLICE] = wflat[c * WSLICE : (c + 1) * WSLICE]
            p[OFF_BU:] = bu[:, q * IBLK : (q + 1) * IBLK].ravel()
    return pack


def _launch(pack):
    fn = _get_runner()
    res = np.asarray(fn(pack))                    # [8*64, 516] int8
    res = res.reshape(NCORES, C_OUT, IBLK + 4)
    q = res[:, :, :IBLK].astype(np.float32)
    rowmax = np.ascontiguousarray(res[:, :, IBLK:]).view(np.float32)  # [8,64,1]
    return q * (rowmax / np.float32(127.0))


def _unpack_out(res, bias):
    out = np.empty((B, C_OUT, N), np.float32)
    for c in range(NCORES):
        b, q = divmod(c, 4)
        out[b][:, q * IBLK : (q + 1) * IBLK] = res[c]
    out += np.asarray(bias, np.float32)[None, :, None]
    return out


def kernel(locs, data, density, weight, bias):
    pack = _prepare_pack(locs, data, density, weight)
    return _unpack_out(_launch(pack), bias)


# -------------------------------------------------------------- benchmarking
def time_kernel(locs, data, density, weight, bias, iters=12):
    """Return (best_wall_s, per_call_s_list) for device launches.

    Host-side input prep runs once outside the loop; each timed iteration
    covers shipping the packed inputs to the 8 cores, executing, fetching,
    and unsharding the output.
    """
    import time

    kernel(locs, data, density, weight, bias)  # warm (compile)
    pack = _prepare_pack(locs, data, density, weight)
    times = []
    for _ in range(iters):
        t0 = time.perf_counter()
        _unpack_out(_launch(pack), bias)
        times.append(time.perf_counter() - t0)
    return min(times), times


# revision 8
# speedup vs baseline: 1.3773x; 1.0408x over previous
"""ConvSP (SPH smoothing-kernel convolution) Trainium2 Bass kernel, v4.

Math (per batch b):
  out[o,i] = bias[o] + sum_k sum_j A_k[o,j] * relu(r2 - |x_i - x_j + off_k|^2)^3
  A_k[o,j] = knorm * sum_c weight[o,c,k] * dcoef[c,j],  dcoef = data / (invmass*density)

Wall-clock is dominated by the axon tunnel: ~40-80ms RTT (time-varying) plus
~75MB/s up / ~45MB/s down. v1 shipped 688KB/core (5.5MB total, payloads
replicated 4-8x). v3 ships each core ~83KB of UNIQUE bytes (666KB total) and
reconstructs everything else on device:

  - dcoef (bf16) + raw positions + tiny coeff tables form a per-batch blob,
    split 4 ways; an on-device HBM AllGather over replica groups
    [[0-3],[4-7]] rebuilds it on every core (NeuronLink, ~us).
  - wT*knorm (bf16, batch-independent) is split 8 ways and AllGathered
    across all cores.
  - t[j,i] = r2 - |x_i - x_j + off_k|^2 is a rank-4 bilinear form
      t = x_j*(2x_i) + y_j*(2y_i) + h_k[j]*1 + 1*row3_k[i]
      h_k[j]   = 2off_k.x_j - n2_j                     (j-side, per k)
      row3_k[i]= r2 - |off_k|^2 - n2_i - 2off_k.x_i    (i-side, per k)
    h/row3 for all 9 cells come from two tiny f32 matmuls against coeff
    tables [3,9]/[4,9]; n2/ones rows are computed on device (ACT square +
    DVE add / memset), so only x,y ship on the j-side. The i-side basis
    [2x_i,2y_i,n2_i] is built ON DEVICE: all 4 group cores assemble the
    full-batch rows prescaled by 1/4 and a ReduceScatter(add) hands rank q
    exactly its i-block (no per-core upload, exact in f32).
  - Output ships int8 [64,512] + per-row f32 abs-max scales (264KB total vs
    1MB f32): on-device DVE abs-max reduce + reciprocal + quantize, host
    dequant. Measured rel err 7.4-7.6e-3 across seeds (gate 2e-2).
  - The jitted executable is AOT-compiled via fast_dispatch_compile (C++
    fast-path dispatch, saves ~2ms/call of Python dispatch overhead).

Device pipeline per core (one batch, one 512-wide i-block):
  A-phase: AT[j,o] per (k, j-chunk) = matmul(dcoef chunk, wT k-block) -> bf16.
  T-phase: one K=4 f32 matmul per [128j x 512i] tile of t; ACT computes t^2,
  DVE w = max(t,0)*t^2 (bf16), PE accumulates out += AT.T @ w over 144 tiles
  into PSUM (two 64-row halves, summed at the end).
"""

import os
import sys

import numpy as np

for _p in ("/opt/trn_rl_repo", "/root/.axon_site/_ro/trn_rl_repo"):
    if os.path.isdir(_p) and _p not in sys.path:
        sys.path.append(_p)

import ml_dtypes  # noqa: E402

import concourse.bass as bass  # noqa: E402
import concourse.mybir as mybir  # noqa: E402
import concourse.tile as tile  # noqa: E402

# ---------------------------------------------------------------- constants
NDIM = 2
KSIZE = (3, 3)
DILATION = (0.05, 0.05)
RADIUS = 0.1
C_IN = 64
C_OUT = 64
B = 2
N = 2048
NCELLS = 9
R2 = RADIUS * RADIUS
KNORM = 315.0 / (64.0 * np.pi * RADIUS**9)

NCORES = 8
IBLK = 512           # i-columns per core
NCHUNK = N // 128    # 16 j-chunks of 128
M_TOT = NCELLS * NCHUNK  # 144 (k, j-chunk) tiles

F32 = mybir.dt.float32
BF16 = mybir.dt.bfloat16

# ---- per-batch blob layout (f32 word offsets; bf16 bit-packed) ----------
SZ_DAT = C_IN * N // 2              # dcoef bf16 [64,2048] -> 65536 words
SZ_POS = 2 * N                      # x row | y row f32    -> 4096
SZ_CH = 3 * NCELLS                  # h coeffs  [3,9]      -> 27
SZ_C4 = 4 * NCELLS                  # row3 coeffs [4,9]    -> 36
OFF_DAT = 0
OFF_POS = OFF_DAT + SZ_DAT
OFF_CH = OFF_POS + SZ_POS
OFF_C4 = OFF_CH + SZ_CH
BLOB_RAW = OFF_C4 + SZ_C4           # 69695
GSLICE = -(-BLOB_RAW // (4 * 16)) * 16  # 17424 words per core (x16 aligned)
BLOB = 4 * GSLICE                   # 69696

SZ_WT = C_IN * NCELLS * C_OUT // 2  # wT bf16 [64,576] -> 18432 words
WSLICE = SZ_WT // NCORES            # 2304 words per core

SZ_BU = 3 * IBLK                    # i-block basis, built on device via RS
PACK = GSLICE + WSLICE              # 19728 f32 words = 77.1 KiB per core
OFF_W = GSLICE

_cache: dict = {}


# ------------------------------------------------- TileContext drain patch
# The walrus in this container rejects the Tile tail-drain when it carries
# more than ~2 sem waits ("Too many sync wait commands"). Split the waits
# over extra sync-engine NOPs, one wait each.
def _patch_tile_drain():
    if getattr(tile.TileContext, "_drain_patched", False):
        return
    import bass_rust
    from concourse.vector_clock import ScopedClock

    def _drain_and_barrier(self, tick_clock, wait_clock):
        drain_inst = self.nc.sync.drain()
        wait_clock.add_sem_waits(
            drain_inst.ins, ScopedClock({None: tick_clock.global_clock})
        )
        si = drain_inst.ins.sync_info
        waits = list(si.on_wait) if si is not None else []
        if len(waits) > 1:
            si.on_wait = waits[:1]
            drain_inst.ins.sync_info = si
            for w in waits[1:]:
                n = self.nc.sync.nop(nofuse=True, hint="drain_wait_split")
                n.ins.sync_info = bass_rust.SyncInfo(on_wait=[w], on_update=[])
        self.nc.all_engine_barrier()
        popped = self.nc._tile_sem_poison_stack.pop()
        assert popped is self._sem_poison
        self.nc.clear_and_free_semaphores(list(self.sems.allocated().values()))
        self.nc.all_engine_barrier()

    tile.TileContext._drain_and_barrier = _drain_and_barrier
    tile.TileContext._drain_patched = True


# --------------------------------------------- sync-wait legalization pass
# This walrus rejects instructions carrying more than ~1-2 sem waits. After
# Tile scheduling, move excess waits onto same-engine NoOps inserted right
# before the over-subscribed instruction (engines execute their stream in
# order, so semantics are identical).
_WAIT_LIMIT = 1


def _split_sync_waits(nc, limit=_WAIT_LIMIT):
    cnt = 0
    for f in nc.m.functions:
        for bb in f.blocks:
            changed = False
            out = []
            for inst in bb.instructions:
                si = inst.sync_info
                waits = list(si.on_wait) if si is not None else []
                if len(waits) > limit:
                    keep = waits[-limit:]
                    excess = waits[:-limit]
                    for j in range(0, len(excess), limit):
                        n = mybir.InstNoOp(
                            name=f"waitsplit_{cnt}",
                            engine=inst.engine,
                            ins=[],
                            outs=[],
                            sync_info=mybir.SyncInfo(
                                on_wait=excess[j : j + limit], on_update=[]
                            ),
                        )
                        cnt += 1
                        nc.register_instruction(n, overwrite=True)
                        out.append(n)
                    si.on_wait = keep
                    inst.sync_info = si
                    changed = True
                out.append(inst)
            if changed:
                bb.instructions = out
    return cnt


# ------------------------------------------------------------- device build
def _build_nc():
    _patch_tile_drain()

    nc = bass.Bass(num_devices=NCORES)
    pack_d = nc.declare_dram_parameter("pack", [1, PACK], F32, isOutput=False)
    out_d = nc.declare_dram_parameter("out", [C_OUT, IBLK + 4], mybir.dt.int8, isOutput=True)

    from contextlib import ExitStack

    with tile.TileContext(nc) as tc, ExitStack() as ctx:
        dram = ctx.enter_context(tc.tile_pool(name="dram", bufs=1, space="DRAM"))
        const = ctx.enter_context(tc.tile_pool(name="const", bufs=1))
        qpool = ctx.enter_context(tc.tile_pool(name="q", bufs=4))
        wpool = ctx.enter_context(tc.tile_pool(name="w", bufs=4))
        apool = ctx.enter_context(tc.tile_pool(name="aps", bufs=2, space="PSUM"))
        tpool = ctx.enter_context(tc.tile_pool(name="t", bufs=2, space="PSUM"))
        opool = ctx.enter_context(tc.tile_pool(name="o", bufs=1, space="PSUM"))

        # ---- AllGather 1: per-batch blob (cores 0-3: b=0, cores 4-7: b=1)
        b_in = dram.tile([1, GSLICE], F32)
        flat = dram.tile([1, BLOB], F32)
        nc.gpsimd.dma_start(b_in[:], pack_d[0:1, 0:GSLICE])
        nc.gpsimd.collective_compute(
            "AllGather",
            mybir.AluOpType.bypass,
            replica_groups=[[0, 1, 2, 3], [4, 5, 6, 7]],
            ins=[b_in[:].opt()],
            outs=[flat[:].opt()],
        )
        # ---- AllGather 2: wT (batch-independent), 8-way
        w_in = dram.tile([1, WSLICE], F32)
        wgat = dram.tile([1, SZ_WT], F32)
        nc.gpsimd.dma_start(w_in[:], pack_d[0:1, OFF_W : OFF_W + WSLICE])
        nc.gpsimd.collective_compute(
            "AllGather",
            mybir.AluOpType.bypass,
            replica_groups=[[0, 1, 2, 3, 4, 5, 6, 7]],
            ins=[w_in[:].opt()],
            outs=[wgat[:].opt()],
        )

        dat_v = (
            flat[0:1, OFF_DAT : OFF_DAT + SZ_DAT]
            .bitcast(BF16)
            .rearrange("a (p c) -> (a p) c", p=C_IN)
        )  # [64, 2048] bf16
        xr_v = flat[0:1, OFF_POS : OFF_POS + N]            # [1, 2048]
        yr_v = flat[0:1, OFF_POS + N : OFF_POS + 2 * N]    # [1, 2048]
        ch_v = flat[0:1, OFF_CH : OFF_CH + SZ_CH].rearrange(
            "a (p c) -> (a p) c", p=3
        )  # [3, 9]
        c4_v = flat[0:1, OFF_C4 : OFF_C4 + SZ_C4].rearrange(
            "a (p c) -> (a p) c", p=4
        )  # [4, 9]
        wt_v = (
            wgat[0:1, :]
            .bitcast(BF16)
            .rearrange("a (p c) -> (a p) c", p=C_IN)
        )  # [64, 576] bf16

        datb = const.tile([C_IN, N], BF16)
        nc.sync.dma_start(datb[:], dat_v)
        wtb = const.tile([C_IN, NCELLS * C_OUT], BF16)
        nc.sync.dma_start(wtb[:], wt_v)

        # j-side basis rows (partition 0 each so ACT/DVE base-alignment holds)
        xr = const.tile([1, N], F32)
        nc.sync.dma_start(xr[:], xr_v)
        yr = const.tile([1, N], F32)
        nc.sync.dma_start(yr[:], yr_v)
        xsq = const.tile([1, N], F32)
        nc.scalar.square(xsq[:], xr[:])
        ysq = const.tile([1, N], F32)
        nc.scalar.square(ysq[:], yr[:])
        n2r = const.tile([1, N], F32)
        nc.vector.tensor_add(n2r[:], xsq[:], ysq[:])
        onesr = const.tile([1, N], F32)
        nc.vector.memset(onesr[:], 1.0)

        # i-block basis [2x,2y,n2]_i for THIS core, with no per-core upload:
        # every core of the group builds the full-batch rows prescaled by
        # 1/4, and a ReduceScatter(add) over the 4 identical contributions
        # hands rank q exactly its i-block (x4 of the 1/4 -- exact in f32).
        x2f = const.tile([1, N], F32)
        nc.vector.tensor_scalar_mul(x2f[:], xr[:], 0.5)   # (2x)/4
        y2f = const.tile([1, N], F32)
        nc.vector.tensor_scalar_mul(y2f[:], yr[:], 0.5)   # (2y)/4
        n2q = const.tile([1, N], F32)
        nc.vector.tensor_scalar_mul(n2q[:], n2r[:], 0.25)  # n2/4
        rs_in = dram.tile([1, 4 * SZ_BU], F32)
        rs_out = dram.tile([1, SZ_BU], F32)
        for q in range(4):
            for ri, srcrow in enumerate((x2f, y2f, n2q)):
                nc.sync.dma_start(
                    rs_in[0:1, q * SZ_BU + ri * IBLK : q * SZ_BU + (ri + 1) * IBLK],
                    srcrow[:, q * IBLK : (q + 1) * IBLK],
                )
        nc.gpsimd.collective_compute(
            "ReduceScatter",
            mybir.AluOpType.add,
            replica_groups=[[0, 1, 2, 3], [4, 5, 6, 7]],
            ins=[rs_in[:].opt()],
            outs=[rs_out[:].opt()],
        )
        bu_v = rs_out[0:1, :].rearrange("a (p c) -> (a p) c", p=3)  # [3,512]

        # matmul operands for the h / row3 coefficient matmuls
        bv_sb = const.tile([3, N], F32)     # [x, y, n2]_j
        nc.sync.dma_start(bv_sb[0:1, :], xr[:])
        nc.sync.dma_start(bv_sb[1:2, :], yr[:])
        nc.sync.dma_start(bv_sb[2:3, :], n2r[:])
        ch_sb = const.tile([3, NCELLS], F32)
        nc.sync.dma_start(ch_sb[:], ch_v)
        c4_sb = const.tile([4, NCELLS], F32)
        nc.sync.dma_start(c4_sb[:], c4_v)
        ub_sb = const.tile([4, IBLK], F32)  # [2x,2y,n2,1]_i basis
        nc.sync.dma_start(ub_sb[0:3, :], bu_v)
        nc.sync.dma_start(ub_sb[3:4, :], onesr[:, 0:IBLK])

        # V/U 4-row groups at 32-aligned partitions (PE tile-position rule).
        # Cell k lives at partition group g=k%4, column block cb=k//4.
        # V rows: [x_j, y_j, h_k[j], 1]   (x/y/1 k-independent, h per-k)
        # U rows: [2x_i, 2y_i, 1, row3_k[i]]
        vt = const.tile([128, 3 * N], F32)
        ut = const.tile([128, 3 * IBLK], F32)
        for k in range(NCELLS):
            g4, cb = 32 * (k % 4), k // 4
            nc.sync.dma_start(vt[g4 : g4 + 1, cb * N : (cb + 1) * N], xr[:])
            nc.sync.dma_start(vt[g4 + 1 : g4 + 2, cb * N : (cb + 1) * N], yr[:])
            nc.sync.dma_start(
                vt[g4 + 3 : g4 + 4, cb * N : (cb + 1) * N], onesr[:]
            )
            nc.sync.dma_start(
                ut[g4 : g4 + 2, cb * IBLK : (cb + 1) * IBLK], bu_v[0:2, :]
            )
            nc.sync.dma_start(
                ut[g4 + 2 : g4 + 3, cb * IBLK : (cb + 1) * IBLK],
                onesr[:, 0:IBLK],
            )

        # h[k, j] for all 9 cells: matmul against [x,y,n2] basis, 4 col chunks
        hstage = const.tile([NCELLS, N], F32)
        for cc in range(4):
            h_ps = opool.tile([NCELLS, 512], F32, tag="ops")
            nc.tensor.matmul(
                h_ps[:], ch_sb[:], bv_sb[:, cc * 512 : (cc + 1) * 512],
                start=True, stop=True,
            )
            nc.scalar.copy(hstage[:, cc * 512 : (cc + 1) * 512], h_ps[:])
        # row3[k, i] for the local i-block: one matmul
        r3stage = const.tile([NCELLS, IBLK], F32)
        r3_ps = opool.tile([NCELLS, IBLK], F32, tag="ops")
        nc.tensor.matmul(r3_ps[:], c4_sb[:], ub_sb[:], start=True, stop=True)
        nc.scalar.copy(r3stage[:], r3_ps[:])
        for k in range(NCELLS):
            g4, cb = 32 * (k % 4), k // 4
            nc.sync.dma_start(
                vt[g4 + 2 : g4 + 3, cb * N : (cb + 1) * N],
                hstage[k : k + 1, :],
            )
            nc.sync.dma_start(
                ut[g4 + 3 : g4 + 4, cb * IBLK : (cb + 1) * IBLK],
                r3stage[k : k + 1, :],
            )

        # A-phase: at[:, m*64:(m+1)*64] = AT chunk for m = k*16 + jc,
        # batched 8 matmuls per PSUM bank -> one ACT copy (f32->bf16) each.
        at = const.tile([128, M_TOT * C_OUT], BF16)
        for g in range(M_TOT // 8):
            a_ps = apool.tile([128, 512], F32)
            for r in range(8):
                m = g * 8 + r
                k, jc = divmod(m, NCHUNK)
                nc.tensor.matmul(
                    a_ps[:, r * 64 : (r + 1) * 64],
                    datb[:, jc * 128 : (jc + 1) * 128],
                    wtb[:, k * 64 : (k + 1) * 64],
                    start=True,
                    stop=True,
                )
            nc.scalar.copy(at[:, g * 512 : (g + 1) * 512], a_ps[:])

        # T-phase + main accumulation, software-pipelined: t-matmuls and
        # ACT/DVE cube for pair p run while PE accumulates mains of p-2.
        out_ps = opool.tile([128, IBLK], F32, tag="ops")
        pend = []  # (m, w_tile, half) awaiting the main matmul
        for gp in range(M_TOT // 2):
            t_ps = tpool.tile([128, 2 * IBLK], F32)
            for r in range(2):
                m = gp * 2 + r
                k, jc = divmod(m, NCHUNK)
                g4, cb = 32 * (k % 4), k // 4
                nc.tensor.matmul(
                    t_ps[:, r * IBLK : (r + 1) * IBLK],
                    vt[g4 : g4 + 4, cb * N + jc * 128 : cb * N + jc * 128 + 128],
                    ut[g4 : g4 + 4, cb * IBLK : (cb + 1) * IBLK],
                    start=True,
                    stop=True,
                    tile_position=(g4, 0),
                )
            q_t = qpool.tile([128, 2 * IBLK], BF16)
            nc.scalar.square(q_t[:], t_ps[:])
            w_t = wpool.tile([128, 2 * IBLK], BF16)
            nc.vector.scalar_tensor_tensor(
                w_t[:], t_ps[:], 0.0, q_t[:],
                op0=mybir.AluOpType.max, op1=mybir.AluOpType.mult,
            )
            pend.append((gp * 2, w_t, 0))
            pend.append((gp * 2 + 1, w_t, 1))
            while len(pend) > 4 or (gp == M_TOT // 2 - 1 and pend):
                m, w_tile, r = pend.pop(0)
                par = (m % 2) * C_OUT
                nc.tensor.matmul(
                    out_ps[par : par + C_OUT, :],
                    at[:, m * C_OUT : (m + 1) * C_OUT],
                    w_tile[:, r * IBLK : (r + 1) * IBLK],
                    start=(m < 2),
                    stop=(m >= M_TOT - 2),
                    skip_group_check=True,
                    tile_position=(0, par),
                )

        tmp_sb = const.tile([C_OUT, IBLK], F32)
        nc.scalar.copy(tmp_sb[:], out_ps[0:C_OUT, :])
        osum = const.tile([C_OUT, IBLK], F32)
        nc.vector.tensor_add(osum[:], tmp_sb[:], out_ps[C_OUT:, :])
        # int8 output: per-row abs-max scale, q = round(osum * 127/rowmax)
        rowmax = const.tile([C_OUT, 1], F32)
        nc.vector.tensor_reduce(
            rowmax[:], osum[:], mybir.AxisListType.X, mybir.AluOpType.max,
            apply_absolute_value=True,
        )
        nc.vector.tensor_scalar_max(rowmax[:], rowmax[:], 1e-20)
        scale = const.tile([C_OUT, 1], F32)
        nc.vector.reciprocal(scale[:], rowmax[:])
        nc.vector.tensor_scalar_mul(scale[:], scale[:], 127.0)
        q8 = const.tile([C_OUT, IBLK], mybir.dt.int8)
        nc.vector.tensor_scalar(
            q8[:], osum[:], scale[:], None, op0=mybir.AluOpType.mult
        )
        nc.sync.dma_start(out_d[:, 0:IBLK], q8[:])
        nc.sync.dma_start(out_d[:, IBLK : IBLK + 4].bitcast(F32), rowmax[:])
    _split_sync_waits(nc)
    return nc


# ------------------------------------------------------------- cached runner
def _get_runner():
    """Build (once) the jitted SPMD executable: pack [8, PACK] -> out bf16.

    Same bass_exec/PJRT machinery as bass_utils.run_bass_kernel_spmd under
    axon, but the executable is cached so repeat launches skip retracing,
    and AOT-compiled under fast_dispatch_compile (C++ fast-path dispatch).
    """
    if "runner" in _cache:
        return _cache["runner"]

    import jax
    from jax.sharding import Mesh, PartitionSpec

    from jax.experimental.shard_map import shard_map
    from concourse import bass2jax

    bass2jax.install_neuronx_cc_hook()

    nc = _build_nc()

    partition_name = (
        nc.partition_id_tensor.name if nc.partition_id_tensor else None
    )
    in_names, out_names, out_avals = [], [], []
    for alloc in nc.m.functions[0].allocations:
        if not isinstance(alloc, mybir.MemoryLocationSet):
            continue
        name = alloc.memorylocations[0].name
        if alloc.kind == "ExternalInput":
            if name != partition_name:
                in_names.append(name)
        elif alloc.kind == "ExternalOutput":
            out_avals.append(
                jax.core.ShapedArray(
                    tuple(alloc.tensor_shape), mybir.dt.np(alloc.dtype)
                )
            )
            out_names.append(name)
    assert in_names == ["pack"] and out_names == ["out"], (in_names, out_names)
    # "out" is deliberately NOT an operand: the kernel writes every element,
    # so no pre-zeroed donated buffer needs to ship through the tunnel.
    in_names_full = in_names + ([partition_name] if partition_name else [])

    def _body(pack):
        operands = [pack]
        if partition_name is not None:
            operands.append(bass2jax.partition_id_tensor())
        outs = bass2jax._bass_exec_p.bind(
            *operands,
            out_avals=tuple(out_avals),
            in_names=tuple(in_names_full),
            out_names=tuple(out_names),
            lowering_input_output_aliases=(),
            sim_require_finite=True,
            sim_require_nnan=True,
            nc=nc,
        )
        return outs[0]

    devices = jax.devices()[:NCORES]
    assert len(devices) == NCORES, f"need {NCORES} devices, got {len(devices)}"
    mesh = Mesh(np.asarray(devices), ("core",))

    def _make_jit():
        return jax.jit(
            shard_map(
                _body,
                mesh=mesh,
                in_specs=(PartitionSpec("core"),),
                out_specs=PartitionSpec("core"),
                check_rep=False,
            )
        )

    try:
        fn = bass2jax.fast_dispatch_compile(
            lambda: _make_jit()
            .lower(jax.ShapeDtypeStruct((NCORES, PACK), np.float32))
            .compile()
        )
    except Exception:
        fn = _make_jit()
    _cache["runner"] = fn
    return fn


# ------------------------------------------------------------ host wrapper
def _offsets():
    axes = [
        (np.arange(kk) - (kk - 1) / 2.0) * d for kk, d in zip(KSIZE, DILATION)
    ]
    grids = np.meshgrid(*axes, indexing="ij")
    return np.stack([g.reshape(-1) for g in grids], axis=-1).astype(np.float32)


def _prepare_pack(locs, data, density, weight):
    locs = np.asarray(locs, np.float32)
    data = np.asarray(data, np.float32)
    density = np.asarray(density, np.float32)
    weight = np.asarray(weight, np.float32)

    pos = locs[..., :NDIM]                        # [B,N,2]
    invmass = locs[..., NDIM]                     # [B,N]
    coef = 1.0 / (invmass * density)              # [B,N]
    dco = (data * coef[:, None, :]).astype(ml_dtypes.bfloat16)  # [B,64,N]
    # wT[c, k*64+o] = weight[o,c,k] * knorm
    wt = np.ascontiguousarray(weight.transpose(1, 2, 0) * np.float32(KNORM))
    wflat = wt.astype(ml_dtypes.bfloat16).reshape(C_IN, -1).view(np.float32)
    wflat = np.ascontiguousarray(wflat).ravel()   # [18432]
    offs = _offsets()                             # [9,2]

    ch = np.empty((3, NCELLS), np.float32)        # h = 2off.x_j - n2_j
    ch[0] = 2.0 * offs[:, 0]
    ch[1] = 2.0 * offs[:, 1]
    ch[2] = -1.0
    c4 = np.empty((4, NCELLS), np.float32)        # row3 on [2x,2y,n2,1]_i
    c4[0] = -offs[:, 0]
    c4[1] = -offs[:, 1]
    c4[2] = -1.0
    c4[3] = R2 - (offs**2).sum(1)

    pack = np.empty((NCORES, PACK), np.float32)
    blob = np.empty(BLOB, np.float32)
    blob[BLOB_RAW:] = 0.0
    for b in range(B):
        x, y = pos[b, :, 0], pos[b, :, 1]
        n2 = x * x + y * y
        blob[OFF_DAT : OFF_DAT + SZ_DAT] = (
            np.ascontiguousarray(dco[b]).view(np.float32).ravel()
        )
        blob[OFF_POS : OFF_POS + N] = x
        blob[OFF_POS + N : OFF_POS + 2 * N] = y
        blob[OFF_CH : OFF_CH + SZ_CH] = ch.ravel()
        blob[OFF_C4 : OFF_C4 + SZ_C4] = c4.ravel()
        for q in range(4):
            c = b * 4 + q
            p = pack[c]
            p[0:GSLICE] = blob[q * GSLICE : (q + 1) * GSLICE]
            p[OFF_W : OFF_W + WSLICE] = wflat[c * WSLICE : (c + 1) * WSLICE]
    return pack


def _launch(pack):
    fn = _get_runner()
    res = np.asarray(fn(pack))                    # [8*64, 516] int8
    res = res.reshape(NCORES, C_OUT, IBLK + 4)
    q = res[:, :, :IBLK].astype(np.float32)
    rowmax = np.ascontiguousarray(res[:, :, IBLK:]).view(np.float32)  # [8,64,1]
    return q * (rowmax / np.float32(127.0))


def _unpack_out(res, bias):
    out = np.empty((B, C_OUT, N), np.float32)
    for c in range(NCORES):
        b, q = divmod(c, 4)
        out[b][:, q * IBLK : (q + 1) * IBLK] = res[c]
    out += np.asarray(bias, np.float32)[None, :, None]
    return out


def kernel(locs, data, density, weight, bias):
    pack = _prepare_pack(locs, data, density, weight)
    return _unpack_out(_launch(pack), bias)


# -------------------------------------------------------------- benchmarking
def time_kernel(locs, data, density, weight, bias, iters=12):
    """Return (best_wall_s, per_call_s_list) for device launches.

    Host-side input prep runs once outside the loop; each timed iteration
    covers shipping the packed inputs to the 8 cores, executing, fetching,
    and unsharding the output.
    """
    import time

    kernel(locs, data, density, weight, bias)  # warm (compile)
    pack = _prepare_pack(locs, data, density, weight)
    times = []
    for _ in range(iters):
        t0 = time.perf_counter()
        _unpack_out(_launch(pack), bias)
        times.append(time.perf_counter() - t0)
    return min(times), times
